# revision 1
# baseline (speedup 1.0000x reference)
"""Trainium2 Bass kernel for MultiHeadGeometryAttention.

Math (per batch b):
  q = x @ Wq + bq ; k = keys @ Wk + bk ; v = values @ Wv + bv   (per-head d=64)
  S_h = q_h k_h^T / 8
  w = softmax(log(clip(g,1e-6)) + where(mask, -inf, S))
    = g * exp(S/8 - 30*mask) / rowsum(...)      (exp(-30) ~ 4e-14 => masked ~ 0)
  out = (w @ v) reshaped @ Wo + bo ; y = LayerNorm(x + out) * gamma + beta

Sharding: 8 cores = 4 batches x 2 query-halves (512 q rows per core).
Each core computes K/V projections for its batch (duplicated between the
2 cores of a pair) and everything else for its q rows. No collectives.

Device dataflow (all layouts chosen so no on-device transposes needed):
  xqT[t](d,nq)   <- strided DMA from queries slice
  QT' = (Wq/240)^T xqT  : [hd, nq]     (scale folded so exp scale=30 later)
  KT  = Wk^T keysT      : [hd, nk]
  Vaug= valuesT^T Wv    : [nk, 16*65]  (per head: 64 v cols + ones col -> rowsum)
  ST'_h = KT_h^T QT'_h  : [nk, nq] in PSUM   (pairs packed via PE row tiling)
  sb  = ST' - mask      (DVE, one op; mask {0,1} bf16)
  pt  = exp(30*sb) * g  (ACT exp -> bf16, DVE mult bf16)
  OT_h/r = (Vaug_h^T pt) : [65, nq], row 64 = rowsum r; divide via PE-replicated 1/r
  Y = OT^T Wo + bo ; + residual ; LayerNorm  (mean/var on DVE/ACT)
"""

import numpy as np
from contextlib import ExitStack

import concourse.bass as bass
import concourse.bacc as bacc
import concourse.tile as tile
from concourse import mybir
from concourse.bass_utils import run_bass_kernel_spmd

P = 128
B, NQ, NK, D, H, DK, DV = 4, 1024, 1024, 1024, 16, 64, 64
NQL = 512           # q rows per core
NCORES = 8
EXPS = 30.0         # exp scale; Wq pre-scaled by 1/(sqrt(64)*EXPS)
QSCALE = 1.0 / (8.0 * EXPS)
LN_EPS = 1e-5

F32 = mybir.dt.float32
F32R = mybir.dt.float32r
BF16 = mybir.dt.bfloat16

DT_KT = 8           # D // P contraction tiles
HT = H * DK // P    # 8 head-dim tiles of 128
KT_TILES = NK // P  # 8 key tiles
QT_TILES = NQL // P # 4 query tiles


def build_nc(stop_after="full", n_pairs=HT, n_kt=KT_TILES, alevel=3, phases="vkqy", ylevel=4):
    nc = bacc.Bacc(None, target_bir_lowering=False)

    xq = nc.dram_tensor("xq", [NQL, D], F32, kind="ExternalInput")
    keys = nc.dram_tensor("keys_in", [NK, D], F32, kind="ExternalInput")
    values = nc.dram_tensor("values_in", [NK, D], F32, kind="ExternalInput")
    g_t = nc.dram_tensor("g_t", [H, NK, NQL], BF16, kind="ExternalInput")
    m_t = nc.dram_tensor("m_t", [H, NK, NQL], BF16, kind="ExternalInput")
    wq = nc.dram_tensor("wq", [D, D], F32, kind="ExternalInput")
    wk = nc.dram_tensor("wk", [D, D], F32, kind="ExternalInput")
    wv = nc.dram_tensor("wv", [D, D], F32, kind="ExternalInput")
    wo = nc.dram_tensor("wo", [D, D], F32, kind="ExternalInput")
    bq = nc.dram_tensor("bq_s", [D], F32, kind="ExternalInput")
    bk = nc.dram_tensor("bk_in", [D], F32, kind="ExternalInput")
    bv = nc.dram_tensor("bv_in", [D], F32, kind="ExternalInput")
    bo = nc.dram_tensor("bo_in", [D], F32, kind="ExternalInput")
    gamma = nc.dram_tensor("gamma_in", [D], F32, kind="ExternalInput")
    beta = nc.dram_tensor("beta_in", [D], F32, kind="ExternalInput")
    ones_in = nc.dram_tensor("ones_in", [P, D], F32R, kind="ExternalInput")
    ones_bf = nc.dram_tensor("ones_bf", [P, H], BF16, kind="ExternalInput")
    gamma2d = nc.dram_tensor("gamma2d", [P, D], F32, kind="ExternalInput")
    beta2d = nc.dram_tensor("beta2d", [P, D], F32, kind="ExternalInput")
    y = nc.dram_tensor("y", [NQL, D], F32, kind="ExternalOutput")

    with tile.TileContext(nc) as tc, ExitStack() as ctx:
        persist = ctx.enter_context(tc.tile_pool(name="persist", bufs=1))

        # ---- constants ----
        ones_row = persist.tile([1, NQL], F32R, tag="ones_row")
        nc.sync.dma_start(out=ones_row, in_=ones_in[0:1, 0:NQL])
        ones_col = persist.tile([P, P], F32R, tag="ones_col")
        nc.sync.dma_start(out=ones_col, in_=ones_in[:, 0:P])
        ones_col32 = persist.tile([P, 64], F32, tag="ones_col32")
        nc.sync.dma_start(out=ones_col32, in_=ones_in[:, 0:64].bitcast(F32))
        bq_sb = persist.tile([1, D], F32R, tag="bq_sb")
        nc.sync.dma_start(out=bq_sb, in_=bq[:].rearrange("(a n) -> a n", a=1).bitcast(F32R))
        bk_sb = persist.tile([1, D], F32R, tag="bk_sb")
        nc.sync.dma_start(out=bk_sb, in_=bk[:].rearrange("(a n) -> a n", a=1).bitcast(F32R))
        bv_sb = persist.tile([1, D], F32R, tag="bv_sb")
        nc.sync.dma_start(out=bv_sb, in_=bv[:].rearrange("(a n) -> a n", a=1).bitcast(F32R))
        bo_sb = persist.tile([1, D], F32R, tag="bo_sb")
        nc.sync.dma_start(out=bo_sb, in_=bo[:].rearrange("(a n) -> a n", a=1).bitcast(F32R))
        # gamma/beta replicated on host
        gamma_b = persist.tile([P, D], F32, tag="gamma_b")
        nc.sync.dma_start(out=gamma_b, in_=gamma2d[:, :])
        beta_b = persist.tile([P, D], F32, tag="beta_b")
        nc.sync.dma_start(out=beta_b, in_=beta2d[:, :])

        # ---- persistent activations ----
        # Vaug: [nk-tile][128, H*65] bf16 (per head: ones col at +64)
        vaug = [persist.tile([P, H * 65], BF16, tag=f"vaug{i}", name=f"vaug{i}")
                for i in range(KT_TILES)]
        # KT: [hd-tile][128, NK] f32
        kt_sb = [persist.tile([P, NK], F32R, tag=f"kt{i}", name=f"kt{i}") for i in range(HT)]
        # QT: [hd-tile][128, NQL] f32
        qt_sb = [persist.tile([P, NQL], F32R, tag=f"qt{i}", name=f"qt{i}") for i in range(HT)]
        # OT (attn out^T): [hd-tile][128, NQL] f32 (head pair t -> rows 0-63/64-127)
        ot_sb = [persist.tile([P, NQL], F32R, tag=f"ot{i}", name=f"ot{i}") for i in range(HT)]

        vT_ap = values[:, :].rearrange("n (t p) -> t p n", p=P)
        kTT_ap = keys[:, :].rearrange("n (t p) -> t p n", p=P)
        xqT_ap = xq[:, :].rearrange("n (t p) -> t p n", p=P)

        # ================= Phase V: Vaug = values^T-proj =================
        if "v" not in phases:
            for i in range(KT_TILES):
                nc.scalar.copy(out=vaug[i][:, 0:D], in_=gamma_b)
                nc.scalar.copy(out=vaug[i][:, D:H * 65], in_=gamma_b[:, 0:H * 65 - D])
        if "v" in phases:
          with tc.tile_pool(name="pv_in", bufs=1) as pv_in, \
             tc.tile_pool(name="pv_w", bufs=1) as pv_w, \
             tc.tile_pool(name="pv_ps", bufs=3, space="PSUM") as pv_ps:
            vT_all = pv_in.tile([P, DT_KT, NK], F32R, name="vT_all")
            wv_all = pv_w.tile([P, DT_KT, D], F32R, name="wv_all")
            for i in range(DT_KT):
                nc.sync.dma_start(out=vT_all[:, i, :], in_=vT_ap[i].bitcast(F32R))
            nc.sync.dma_start(out=wv_all, in_=wv[:, :].rearrange("(t p) n -> p t n", p=P).bitcast(F32R))
            vT = [vT_all[:, i, :] for i in range(DT_KT)]
            wv_sb = [wv_all[:, i, :] for i in range(DT_KT)]
            for i in range(KT_TILES):
                nc.sync.dma_start(
                    out=vaug[i].rearrange("p (h c) -> p h c", c=65)[:, :, 64:65],
                    in_=ones_bf[:, :, None])
                for half in range(2):
                    ps = pv_ps.tile([P, 512], F32, tag="vps")
                    for dt in range(DT_KT):
                        nc.tensor.matmul(
                            ps,
                            lhsT=vT[dt][:, i * P:(i + 1) * P],
                            rhs=wv_sb[dt][:, half * 512:(half + 1) * 512],
                            start=(dt == 0), stop=False)
                    nc.tensor.matmul(  # + bv (rank-1)
                        ps,
                        lhsT=ones_col[0:1, :],
                        rhs=bv_sb[0:1, half * 512:(half + 1) * 512],
                        start=False, stop=True)
                    nc.scalar.copy(
                        out=vaug[i].rearrange("p (h c) -> p h c", c=65)
                            [:, half * 8:(half + 1) * 8, 0:64],
                        in_=ps.rearrange("p (h c) -> p h c", c=64))

        # ================= Phase K: KT = Wk^T keysT =================
        if "k" not in phases:
            for i in range(HT):
                nc.sync.dma_start(out=kt_sb[i], in_=ones_in[:, 0:NK])
        if "k" in phases:
          with tc.tile_pool(name="pk_in", bufs=1) as pk_in, \
             tc.tile_pool(name="pk_w", bufs=1) as pk_w, \
             tc.tile_pool(name="pk_ps", bufs=3, space="PSUM") as pk_ps:
            kT_all = pk_in.tile([P, DT_KT, NK], F32R, name="kT_all")
            wk_all = pk_w.tile([P, DT_KT, D], F32R, name="wk_all")
            for i in range(DT_KT):
                nc.sync.dma_start(out=kT_all[:, i, :], in_=kTT_ap[i].bitcast(F32R))
            nc.sync.dma_start(out=wk_all, in_=wk[:, :].rearrange("(t p) n -> p t n", p=P).bitcast(F32R))
            kT = [kT_all[:, i, :] for i in range(DT_KT)]
            wk_sb = [wk_all[:, i, :] for i in range(DT_KT)]
            for ht in range(HT):
                for half in range(2):
                    ps = pk_ps.tile([P, 512], F32, tag="kps")
                    for dt in range(DT_KT):
                        nc.tensor.matmul(
                            ps,
                            lhsT=wk_sb[dt][:, ht * P:(ht + 1) * P],
                            rhs=kT[dt][:, half * 512:(half + 1) * 512],
                            start=(dt == 0), stop=False)
                    nc.tensor.matmul(  # + bk (rank-1)
                        ps,
                        lhsT=bk_sb[0:1, ht * P:(ht + 1) * P],
                        rhs=ones_row[0:1, 0:512],
                        start=False, stop=True)
                    nc.scalar.copy(
                        out=kt_sb[ht][:, half * 512:(half + 1) * 512], in_=ps)

        # ================= Phase Q: QT' = (Wq/240)^T xqT =================
        if "q" not in phases:
            for i in range(HT):
                nc.sync.dma_start(out=qt_sb[i], in_=ones_in[:, 0:NQL])
        if "q" in phases:
          with tc.tile_pool(name="pq_in", bufs=1) as pq_in, \
             tc.tile_pool(name="pq_w", bufs=1) as pq_w, \
             tc.tile_pool(name="pq_ps", bufs=3, space="PSUM") as pq_ps:
            xqT_all = pq_in.tile([P, DT_KT, NQL], F32R, name="xqT_all")
            wq_all = pq_w.tile([P, DT_KT, D], F32R, name="wq_all")
            for i in range(DT_KT):
                nc.sync.dma_start(out=xqT_all[:, i, :], in_=xqT_ap[i].bitcast(F32R))
            nc.sync.dma_start(out=wq_all, in_=wq[:, :].rearrange("(t p) n -> p t n", p=P).bitcast(F32R))
            xqT = [xqT_all[:, i, :] for i in range(DT_KT)]
            wq_sb = [wq_all[:, i, :] for i in range(DT_KT)]
            for ht in range(HT):
                ps = pq_ps.tile([P, NQL], F32, tag="qps")
                for dt in range(DT_KT):
                    nc.tensor.matmul(
                        ps,
                        lhsT=wq_sb[dt][:, ht * P:(ht + 1) * P],
                        rhs=xqT[dt],
                        start=(dt == 0), stop=False)
                nc.tensor.matmul(  # + bq/240 (rank-1)
                    ps,
                    lhsT=bq_sb[0:1, ht * P:(ht + 1) * P],
                    rhs=ones_row[0:1, :],
                    start=False, stop=True)
                nc.scalar.copy(out=qt_sb[ht], in_=ps)

        if stop_after == "proj":
            for i in range(4):
                nc.sync.dma_start(out=y[i * P:(i + 1) * P, :],
                                  in_=kt_sb[i].bitcast(F32))
            nc.compile()
            return nc

        # ================= Phase A: attention =================
        with tc.tile_pool(name="pa_gm", bufs=4) as pa_gm, \
             tc.tile_pool(name="pa_sb", bufs=3) as pa_sb, \
             tc.tile_pool(name="pa_pt", bufs=3) as pa_pt, \
             tc.tile_pool(name="pa_ep", bufs=2) as pa_ep, \
             tc.tile_pool(name="pa_st", bufs=3, space="PSUM") as pa_st, \
             tc.tile_pool(name="pa_ot", bufs=2, space="PSUM") as pa_ot, \
             tc.tile_pool(name="pa_rb", bufs=1, space="PSUM") as pa_rb:
            if alevel < 3:
                for i in range(HT):
                    nc.sync.dma_start(out=ot_sb[i], in_=ones_in[:, 0:NQL])
            for t in range(n_pairs):  # head pair (2t, 2t+1)
                if alevel == 0:
                    break
                otp = [pa_ot.tile([65, NQL], F32, tag=f"otps{j}", name=f"otps{j}") for j in range(2)]
                for kt in range(n_kt):
                    for j in range(2):
                        h = 2 * t + j
                        stp = pa_st.tile([P, NQL], F32, tag="stps")
                        # ST' = KT_h^T QT'_h  (head pair packs PE rows 0-63/64-127)
                        nc.tensor.matmul(
                            stp,
                            lhsT=kt_sb[t][64 * j:64 * j + 64,
                                              kt * P:(kt + 1) * P],
                            rhs=qt_sb[t][64 * j:64 * j + 64, :],
                            start=True, stop=True)
                        mt = pa_gm.tile([P, NQL], BF16, tag="mt")
                        nc.scalar.dma_start(out=mt, in_=m_t[h, kt * P:(kt + 1) * P, :])
                        gt = pa_gm.tile([P, NQL], BF16, tag="gt")
                        nc.scalar.dma_start(out=gt, in_=g_t[h, kt * P:(kt + 1) * P, :])
                        sb = pa_sb.tile([P, NQL], F32, tag="sb")
                        nc.vector.tensor_tensor(
                            out=sb, in0=stp, in1=mt, op=mybir.AluOpType.subtract)
                        pt0 = pa_pt.tile([P, NQL], BF16, tag="pt0")
                        nc.scalar.activation(
                            out=pt0, in_=sb,
                            func=mybir.ActivationFunctionType.Exp, scale=EXPS)
                        pt = pa_pt.tile([P, NQL], BF16, tag="pt")
                        nc.vector.tensor_tensor(
                            out=pt, in0=pt0, in1=gt, op=mybir.AluOpType.mult)
                        if alevel < 2:
                            continue
                        # OT_h (+rowsum r in row 64) += Vaug_h^T pt
                        nc.tensor.matmul(
                            otp[j],
                            lhsT=vaug[kt][:, h * 65:(h + 1) * 65],
                            rhs=pt,
                            start=(kt == 0), stop=(kt == n_kt - 1))
                if alevel < 2:
                    continue
                if alevel == 2:
                    for j in range(2):
                        nc.scalar.copy(out=ot_sb[t][0:64, :].bitcast(F32),
                                       in_=otp[j][0:64, :])
                    continue
                # epilogue: divide by rowsum, store into ot_sb[t]
                for j in range(2):
                    rinv = pa_ep.tile([P, NQL], F32, tag="rinv")
                    nc.vector.reciprocal(out=rinv[64:65, :], in_=otp[j][64:65, :])
                    rb = pa_rb.tile([64, NQL], F32, tag="rb")
                    nc.tensor.matmul(
                        rb,
                        lhsT=ones_col32[64:65, :],
                        rhs=rinv[64:65, :],
                        start=True, stop=True)
                    rb_sb = pa_ep.tile([64, NQL], F32, tag="rb_sb")
                    nc.scalar.copy(out=rb_sb, in_=rb)
                    if j == 0:
                        nc.vector.tensor_tensor(
                            out=ot_sb[t][0:64, :], in0=otp[j][0:64, :], in1=rb_sb,
                            op=mybir.AluOpType.mult)
                    else:
                        tmp = pa_ep.tile([64, NQL], F32R, tag="ottmp")
                        nc.vector.tensor_tensor(
                            out=tmp, in0=otp[j][0:64, :], in1=rb_sb,
                            op=mybir.AluOpType.mult)
                        # partition shift 0-63 -> 64-127 needs a DMA hop
                        nc.gpsimd.dma_start(out=ot_sb[t][64:128, :], in_=tmp)

        if stop_after == "attn":
            for i in range(HT):
                nc.sync.dma_start(out=y[i * 64:(i + 1) * 64, 0:NQL],
                                  in_=ot_sb[i][0:64, :].bitcast(F32))
            nc.compile()
            return nc

        # ================= Phase Y: out proj + residual + LN =================
        if "y" not in phases:
            for qt in range(QT_TILES):
                nc.sync.dma_start(out=y[qt * P:(qt + 1) * P, :],
                                  in_=ones_in[:, :].bitcast(F32))
        if "y" in phases:
          with tc.tile_pool(name="py_w", bufs=1) as py_w, \
             tc.tile_pool(name="py_x", bufs=2) as py_x, \
             tc.tile_pool(name="py_t", bufs=2) as py_t, \
             tc.tile_pool(name="py_s", bufs=4) as py_s, \
             tc.tile_pool(name="py_ps", bufs=2, space="PSUM") as py_ps:
            wo_all = py_w.tile([P, HT, D], F32R, name="wo_all")
            nc.sync.dma_start(out=wo_all, in_=wo[:, :].rearrange("(t p) n -> p t n", p=P).bitcast(F32R))
            wo_sb = [wo_all[:, i, :] for i in range(HT)]
            for qt in range(QT_TILES):
                xres = py_x.tile([P, D], F32, tag="xres")
                nc.sync.dma_start(out=xres, in_=xq[qt * P:(qt + 1) * P, :])
                yps = py_ps.tile([P, D], F32, tag="yps")
                for half in range(2):
                    for ht in range(HT):
                        nc.tensor.matmul(
                            yps[:, half * 512:(half + 1) * 512],
                            lhsT=ot_sb[ht][:, qt * P:(qt + 1) * P],
                            rhs=wo_sb[ht][:, half * 512:(half + 1) * 512],
                            start=(ht == 0), stop=False)
                    nc.tensor.matmul(  # + bo (rank-1)
                        yps[:, half * 512:(half + 1) * 512],
                        lhsT=ones_col[0:1, 0:P],
                        rhs=bo_sb[0:1, half * 512:(half + 1) * 512],
                        start=False, stop=True)
                if ylevel == 1:
                    ycp = py_t.tile([P, D], F32, tag="ycp")
                    nc.scalar.copy(out=ycp, in_=yps)
                    nc.sync.dma_start(out=y[qt * P:(qt + 1) * P, :], in_=ycp)
                    continue
                # residual add (psum + sbuf -> sbuf)
                x_t = py_t.tile([P, D], F32, tag="x_t")
                nc.vector.tensor_tensor(
                    out=x_t, in0=yps, in1=xres, op=mybir.AluOpType.add)
                if ylevel == 2:
                    nc.sync.dma_start(out=y[qt * P:(qt + 1) * P, :], in_=x_t)
                    continue
                # mean/var in one pass via bn_stats/bn_aggr
                nsub = D // nc.vector.BN_STATS_FMAX
                stats = py_s.tile([P, nsub, nc.vector.BN_STATS_DIM], F32,
                                  tag="stats")
                xg = x_t.rearrange("p (s f) -> p s f", s=nsub)
                for s in range(nsub):
                    nc.vector.bn_stats(out=stats[:, s, :], in_=xg[:, s, :])
                mv = py_s.tile([P, nc.vector.BN_AGGR_DIM], F32, tag="mv")
                nc.vector.bn_aggr(out=mv, in_=stats)
                var_eps = py_s.tile([P, 1], F32, tag="var_eps")
                nc.vector.tensor_scalar(
                    out=var_eps, in0=mv[:, 1:2], scalar1=LN_EPS, scalar2=None,
                    op0=mybir.AluOpType.add)
                rvar = py_s.tile([P, 1], F32, tag="rvar")
                nc.vector.reciprocal(out=rvar, in_=var_eps)
                rstd = py_s.tile([P, 1], F32, tag="rstd")
                nc.scalar.sqrt(out=rstd, in_=rvar)
                xhat = py_t.tile([P, D], F32, tag="xhat")
                nc.vector.tensor_scalar(
                    out=xhat, in0=x_t, scalar1=mv[:, 0:1], scalar2=rstd,
                    op0=mybir.AluOpType.subtract, op1=mybir.AluOpType.mult)
                if ylevel == 3:
                    nc.sync.dma_start(out=y[qt * P:(qt + 1) * P, :], in_=xhat)
                    continue
                yout = py_t.tile([P, D], F32, tag="yout")
                nc.vector.tensor_tensor(
                    out=yout, in0=xhat, in1=gamma_b, op=mybir.AluOpType.mult)
                nc.vector.tensor_tensor(
                    out=yout, in0=yout, in1=beta_b, op=mybir.AluOpType.add)
                nc.sync.dma_start(out=y[qt * P:(qt + 1) * P, :], in_=yout)

    nc.compile()
    return nc


_NC_CACHE = {}


def _get_nc():
    if "nc" not in _NC_CACHE:
        _NC_CACHE["nc"] = build_nc()
    return _NC_CACHE["nc"]


def make_in_maps(queries, keys, values, geometry, attention_mask,
                 Wq, bq, Wk, bk, Wv, bv, Wo, bo, ln_gamma, ln_beta):
    bf16 = mybir.dt.np(BF16)
    f32 = np.float32
    wq_s = np.ascontiguousarray(Wq, dtype=f32) * np.float32(QSCALE)
    bq_s = np.ascontiguousarray(bq, dtype=f32) * np.float32(QSCALE)
    shared = {
        "wq": wq_s,
        "wk": np.ascontiguousarray(Wk, dtype=f32),
        "wv": np.ascontiguousarray(Wv, dtype=f32),
        "wo": np.ascontiguousarray(Wo, dtype=f32),
        "bq_s": bq_s,
        "bk_in": np.ascontiguousarray(bk, dtype=f32),
        "bv_in": np.ascontiguousarray(bv, dtype=f32),
        "bo_in": np.ascontiguousarray(bo, dtype=f32),
        "gamma_in": np.ascontiguousarray(ln_gamma, dtype=f32),
        "beta_in": np.ascontiguousarray(ln_beta, dtype=f32),
        "ones_in": np.ones((P, D), dtype=f32),
        "ones_bf": np.ones((P, H), dtype=bf16),
        "gamma2d": np.broadcast_to(np.ascontiguousarray(ln_gamma, dtype=f32), (P, D)).copy(),
        "beta2d": np.broadcast_to(np.ascontiguousarray(ln_beta, dtype=f32), (P, D)).copy(),
    }
    in_maps = []
    for c in range(NCORES):
        b, qh = c // 2, c % 2
        qs = slice(qh * NQL, (qh + 1) * NQL)
        g_slice = geometry[b, :, qs, :]           # [H, NQL, NK]
        m_slice = attention_mask[b, :, qs, :]
        in_maps.append({
            "xq": np.ascontiguousarray(queries[b, qs], dtype=f32),
            "keys_in": np.ascontiguousarray(keys[b], dtype=f32),
            "values_in": np.ascontiguousarray(values[b], dtype=f32),
            "g_t": np.ascontiguousarray(
                g_slice.transpose(0, 2, 1).astype(bf16)),
            "m_t": np.ascontiguousarray(
                m_slice.transpose(0, 2, 1).astype(bf16)),
            **shared,
        })
    return in_maps


def kernel(queries, keys, values, geometry, attention_mask,
           Wq, bq, Wk, bk, Wv, bv, Wo, bo, ln_gamma, ln_beta, **run_kwargs):
    nc = _get_nc()
    in_maps = make_in_maps(queries, keys, values, geometry, attention_mask,
                           Wq, bq, Wk, bk, Wv, bv, Wo, bo, ln_gamma, ln_beta)
    res = run_bass_kernel_spmd(nc, in_maps, core_ids=list(range(NCORES)),
                               **run_kwargs)
    out = np.empty((B, NQ, D), np.float32)
    for c in range(NCORES):
        b, qh = c // 2, c % 2
        out[b, qh * NQL:(qh + 1) * NQL, :] = res.results[c]["y"]
    if run_kwargs:
        kernel.last_results = res
    return out



# revision 10
# speedup vs baseline: 4.8372x; 4.8372x over previous
"""Trainium2 Bass kernel for MultiHeadGeometryAttention.

Math (per batch b):
  q = x @ Wq + bq ; k = keys @ Wk + bk ; v = values @ Wv + bv   (per-head d=64)
  S_h = q_h k_h^T / 8
  w = softmax(log(clip(g,1e-6)) + where(mask, -inf, S))
    = g_eff * exp(S/8) / rowsum(...)    with g_eff = where(mask, 0, clip(g,1e-6))
  out = (w @ v) reshaped @ Wo + bo ; y = LayerNorm(x + out) * gamma + beta

Sharding: 8 cores = 4 batches x 2 query-halves (512 q rows per core).
Each core computes K/V projections for its batch (duplicated between the
2 cores of a pair) and everything else for its q rows. No collectives.

All DRAM inputs are pre-packed on the host so that every SBUF tile load is
a large contiguous block (the transposes happen in numpy, not in 4-byte
strided DMA descriptors).

Device dataflow:
  xqT[t](128,nq)  <- contiguous tiles of queries-slice^T (host packed)
  QT' = (Wq/240)^T xqT  : [hd, nq]     (scale folded so exp scale=30 later)
  KT  = Wk^T keysT      : [hd, nk]
  Vaug= valuesT^T Wv    : [nk, 16*65]  (per head: 64 v cols + ones col -> rowsum)
  ST'_h = KT_h^T QT'_h  : [nk, nq] in PSUM   (pairs packed via PE row tiling)
  pt  = exp(30*ST') * g_eff  (ACT exp from PSUM -> bf16, DVE mult bf16)
  OT_h/r = (Vaug_h^T pt) : [65, nq], row 64 = rowsum r; divide via PE-replicated 1/r
  Y = OT^T Wo + bo ; + residual ; LayerNorm  (mean/var on DVE/ACT)
"""

import numpy as np
from contextlib import ExitStack

import concourse.bass as bass
import concourse.bacc as bacc
import concourse.tile as tile
from concourse import mybir
from concourse.bass_utils import run_bass_kernel_spmd

P = 128
B, NQ, NK, D, H, DK, DV = 4, 1024, 1024, 1024, 16, 64, 64
NQL = 512           # q rows per core
NCORES = 8
EXPS = 30.0         # exp scale; Wq pre-scaled by 1/(sqrt(64)*EXPS)
QSCALE = 1.0 / (8.0 * EXPS)
LN_EPS = 1e-5

F32 = mybir.dt.float32
F32R = mybir.dt.float32r
BF16 = mybir.dt.bfloat16

DT_KT = 8           # D // P contraction tiles
HT = H * DK // P    # 8 head-dim tiles of 128
KT_TILES = NK // P  # 8 key tiles
QT_TILES = NQL // P # 4 query tiles


def build_nc():
    import os
    safe = set(os.environ.get("KVAR", "").split(","))
    nc = bacc.Bacc(None, target_bir_lowering=False)

    # host-packed layouts: every tile load below is contiguous in DRAM
    xqT = nc.dram_tensor("xqT", [DT_KT, P, NQL], F32, kind="ExternalInput")
    keysT = nc.dram_tensor("keysT", [DT_KT, P, NK], F32, kind="ExternalInput")
    valuesT = nc.dram_tensor("valuesT", [DT_KT, P, NK], F32, kind="ExternalInput")
    xq = nc.dram_tensor("xq", [NQL, D], F32, kind="ExternalInput")
    g_t = nc.dram_tensor("g_t", [H, KT_TILES, P, NQL], BF16, kind="ExternalInput")
    wq = nc.dram_tensor("wq", [P, DT_KT, D], F32, kind="ExternalInput")
    wk = nc.dram_tensor("wk", [P, DT_KT, D], F32, kind="ExternalInput")
    wv = nc.dram_tensor("wv", [P, DT_KT, D], F32, kind="ExternalInput")
    wo = nc.dram_tensor("wo", [P, HT, D], F32, kind="ExternalInput")
    bq = nc.dram_tensor("bq_s", [D], F32, kind="ExternalInput")
    bk = nc.dram_tensor("bk_in", [D], F32, kind="ExternalInput")
    bv = nc.dram_tensor("bv_in", [D], F32, kind="ExternalInput")
    bo = nc.dram_tensor("bo_in", [D], F32, kind="ExternalInput")
    ones_in = nc.dram_tensor("ones_in", [P, D], F32R, kind="ExternalInput")
    ones_bf = nc.dram_tensor("ones_bf", [P, H], BF16, kind="ExternalInput")
    gamma2d = nc.dram_tensor("gamma2d", [P, D], F32, kind="ExternalInput")
    beta2d = nc.dram_tensor("beta2d", [P, D], F32, kind="ExternalInput")
    y = nc.dram_tensor("y", [NQL, D], F32, kind="ExternalOutput")

    with tile.TileContext(nc) as tc, ExitStack() as ctx:
        persist = ctx.enter_context(tc.tile_pool(name="persist", bufs=1))

        # ---- constants ----
        ones_row = persist.tile([1, NQL], F32R, tag="ones_row")
        nc.sync.dma_start(out=ones_row, in_=ones_in[0:1, 0:NQL])
        ones_col = persist.tile([P, P], F32R, tag="ones_col")
        nc.sync.dma_start(out=ones_col, in_=ones_in[:, 0:P])
        ones_col32 = persist.tile([P, 64], F32, tag="ones_col32")
        nc.sync.dma_start(out=ones_col32, in_=ones_in[:, 0:64].bitcast(F32))
        ones_hb = persist.tile([P, H], BF16, tag="ones_hb")
        nc.sync.dma_start(out=ones_hb, in_=ones_bf[:, :])
        bq_sb = persist.tile([1, D], F32R, tag="bq_sb")
        nc.sync.dma_start(out=bq_sb, in_=bq[:].rearrange("(a n) -> a n", a=1).bitcast(F32R))
        bk_sb = persist.tile([1, D], F32R, tag="bk_sb")
        nc.sync.dma_start(out=bk_sb, in_=bk[:].rearrange("(a n) -> a n", a=1).bitcast(F32R))
        bv_sb = persist.tile([1, D], F32R, tag="bv_sb")
        nc.sync.dma_start(out=bv_sb, in_=bv[:].rearrange("(a n) -> a n", a=1).bitcast(F32R))
        bo_sb = persist.tile([1, D], F32R, tag="bo_sb")
        nc.sync.dma_start(out=bo_sb, in_=bo[:].rearrange("(a n) -> a n", a=1).bitcast(F32R))
        # gamma/beta replicated on host
        gamma_b = persist.tile([P, D], F32, tag="gamma_b")
        nc.sync.dma_start(out=gamma_b, in_=gamma2d[:, :])
        beta_b = persist.tile([P, D], F32, tag="beta_b")
        nc.sync.dma_start(out=beta_b, in_=beta2d[:, :])

        # ---- persistent activations ----
        # Vaug: [nk-tile][128, H*65] bf16 (per head: ones col at +64)
        vaug = [persist.tile([P, H * 65], BF16, tag=f"vaug{i}", name=f"vaug{i}")
                for i in range(KT_TILES)]
        # KT: [hd-tile][128, NK] f32
        kt_sb = [persist.tile([P, NK], F32R, tag=f"kt{i}", name=f"kt{i}") for i in range(HT)]
        # QT: [hd-tile][128, NQL] f32
        qt_sb = [persist.tile([P, NQL], F32R, tag=f"qt{i}", name=f"qt{i}") for i in range(HT)]
        # OT (attn out^T): [hd-tile][128, NQL] f32 (head pair t -> rows 0-63/64-127)
        ot_sb = [persist.tile([P, NQL], F32R, tag=f"ot{i}", name=f"ot{i}") for i in range(HT)]

        # ================= Phase V: Vaug = values^T-proj =================
        with tc.tile_pool(name="pv_in", bufs=1) as pv_in, \
             tc.tile_pool(name="pv_w", bufs=1) as pv_w, \
             tc.tile_pool(name="pv_ps", bufs=3, space="PSUM") as pv_ps:
            vT_all = pv_in.tile([P, DT_KT, NK], F32R, name="vT_all")
            wv_all = pv_w.tile([P, DT_KT, D], F32R, name="wv_all")
            for i in range(DT_KT):
                nc.sync.dma_start(out=vT_all[:, i, :], in_=valuesT[i].bitcast(F32R))
            nc.sync.dma_start(out=wv_all, in_=wv[:, :, :].bitcast(F32R))
            vT = [vT_all[:, i, :] for i in range(DT_KT)]
            wv_sb = [wv_all[:, i, :] for i in range(DT_KT)]
            for i in range(KT_TILES):
                if "dmaones" in safe:
                    nc.sync.dma_start(
                        out=vaug[i].rearrange("p (h c) -> p h c", c=65)[:, :, 64:65],
                        in_=ones_bf[:, :, None])
                else:
                    nc.scalar.copy(
                        out=vaug[i].rearrange("p (h c) -> p h c", c=65)[:, :, 64:65],
                        in_=ones_hb[:, :, None])
                for half in range(2):
                    ps = pv_ps.tile([P, 512], F32, tag="vps")
                    for dt in range(DT_KT):
                        nc.tensor.matmul(
                            ps,
                            lhsT=vT[dt][:, i * P:(i + 1) * P],
                            rhs=wv_sb[dt][:, half * 512:(half + 1) * 512],
                            start=(dt == 0), stop=False)
                    nc.tensor.matmul(  # + bv (rank-1)
                        ps,
                        lhsT=ones_col[0:1, :],
                        rhs=bv_sb[0:1, half * 512:(half + 1) * 512],
                        start=False, stop=True)
                    nc.scalar.copy(
                        out=vaug[i].rearrange("p (h c) -> p h c", c=65)
                            [:, half * 8:(half + 1) * 8, 0:64],
                        in_=ps.rearrange("p (h c) -> p h c", c=64))

        # ================= Phase K: KT = Wk^T keysT =================
        with tc.tile_pool(name="pk_in", bufs=1) as pk_in, \
             tc.tile_pool(name="pk_w", bufs=1) as pk_w, \
             tc.tile_pool(name="pk_ps", bufs=3, space="PSUM") as pk_ps:
            kT_all = pk_in.tile([P, DT_KT, NK], F32R, name="kT_all")
            wk_all = pk_w.tile([P, DT_KT, D], F32R, name="wk_all")
            for i in range(DT_KT):
                nc.sync.dma_start(out=kT_all[:, i, :], in_=keysT[i].bitcast(F32R))
            nc.sync.dma_start(out=wk_all, in_=wk[:, :, :].bitcast(F32R))
            kT = [kT_all[:, i, :] for i in range(DT_KT)]
            wk_sb = [wk_all[:, i, :] for i in range(DT_KT)]
            for ht in range(HT):
                for half in range(2):
                    ps = pk_ps.tile([P, 512], F32, tag="kps")
                    for dt in range(DT_KT):
                        nc.tensor.matmul(
                            ps,
                            lhsT=wk_sb[dt][:, ht * P:(ht + 1) * P],
                            rhs=kT[dt][:, half * 512:(half + 1) * 512],
                            start=(dt == 0), stop=False)
                    nc.tensor.matmul(  # + bk (rank-1)
                        ps,
                        lhsT=bk_sb[0:1, ht * P:(ht + 1) * P],
                        rhs=ones_row[0:1, 0:512],
                        start=False, stop=True)
                    nc.scalar.copy(
                        out=kt_sb[ht][:, half * 512:(half + 1) * 512], in_=ps)

        # ================= Phase Q: QT' = (Wq/240)^T xqT =================
        with tc.tile_pool(name="pq_in", bufs=1) as pq_in, \
             tc.tile_pool(name="pq_w", bufs=1) as pq_w, \
             tc.tile_pool(name="pq_ps", bufs=3, space="PSUM") as pq_ps:
            xqT_all = pq_in.tile([P, DT_KT, NQL], F32R, name="xqT_all")
            wq_all = pq_w.tile([P, DT_KT, D], F32R, name="wq_all")
            for i in range(DT_KT):
                nc.sync.dma_start(out=xqT_all[:, i, :], in_=xqT[i].bitcast(F32R))
            nc.sync.dma_start(out=wq_all, in_=wq[:, :, :].bitcast(F32R))
            xqT_sb = [xqT_all[:, i, :] for i in range(DT_KT)]
            wq_sb = [wq_all[:, i, :] for i in range(DT_KT)]
            for ht in range(HT):
                ps = pq_ps.tile([P, NQL], F32, tag="qps")
                for dt in range(DT_KT):
                    nc.tensor.matmul(
                        ps,
                        lhsT=wq_sb[dt][:, ht * P:(ht + 1) * P],
                        rhs=xqT_sb[dt],
                        start=(dt == 0), stop=False)
                nc.tensor.matmul(  # + bq/240 (rank-1)
                    ps,
                    lhsT=bq_sb[0:1, ht * P:(ht + 1) * P],
                    rhs=ones_row[0:1, :],
                    start=False, stop=True)
                nc.scalar.copy(out=qt_sb[ht], in_=ps)

        # ================= Phase A: attention =================
        with tc.tile_pool(name="pa_gm", bufs=6) as pa_gm, \
             tc.tile_pool(name="pa_pt", bufs=3) as pa_pt, \
             tc.tile_pool(name="pa_ep", bufs=2) as pa_ep, \
             tc.tile_pool(name="pa_st", bufs=3, space="PSUM") as pa_st, \
             tc.tile_pool(name="pa_ot", bufs=2, space="PSUM") as pa_ot, \
             tc.tile_pool(name="pa_rb", bufs=1, space="PSUM") as pa_rb:
            for t in range(HT):  # head pair (2t, 2t+1)
                otp = [pa_ot.tile([65, NQL], F32, tag=f"otps{j}", name=f"otps{j}") for j in range(2)]
                for kt in range(KT_TILES):
                    for j in range(2):
                        h = 2 * t + j
                        stp = pa_st.tile([P, NQL], F32, tag="stps")
                        # ST' = KT_h^T QT'_h  (head pair packs PE rows 0-63/64-127)
                        nc.tensor.matmul(
                            stp,
                            lhsT=kt_sb[t][64 * j:64 * j + 64,
                                              kt * P:(kt + 1) * P],
                            rhs=qt_sb[t][64 * j:64 * j + 64, :],
                            start=True, stop=True)
                        gt = pa_gm.tile([P, NQL], BF16, tag="gt")
                        if "scalardma" in safe:
                            nc.scalar.dma_start(out=gt, in_=g_t[h, kt])
                        else:
                            nc.gpsimd.dma_start(out=gt, in_=g_t[h, kt])
                        pt0 = pa_pt.tile([P, NQL], BF16, tag="pt0")
                        if "sbexp" in safe:
                            sb = pa_pt.tile([P, NQL], F32, tag="sbx")
                            nc.vector.tensor_scalar(
                                out=sb, in0=stp, scalar1=1.0, scalar2=None,
                                op0=mybir.AluOpType.mult)
                            nc.scalar.activation(
                                out=pt0, in_=sb,
                                func=mybir.ActivationFunctionType.Exp, scale=EXPS)
                        else:
                            nc.scalar.activation(
                                out=pt0, in_=stp,
                                func=mybir.ActivationFunctionType.Exp, scale=EXPS)
                        pt = pa_pt.tile([P, NQL], BF16, tag="pt")
                        nc.vector.tensor_tensor(
                            out=pt, in0=pt0, in1=gt, op=mybir.AluOpType.mult)
                        # OT_h (+rowsum r in row 64) += Vaug_h^T pt
                        nc.tensor.matmul(
                            otp[j],
                            lhsT=vaug[kt][:, h * 65:(h + 1) * 65],
                            rhs=pt,
                            start=(kt == 0), stop=(kt == KT_TILES - 1))
                # epilogue: divide by rowsum, store into ot_sb[t]
                for j in range(2):
                    rinv = pa_ep.tile([P, NQL], F32, tag="rinv")
                    nc.vector.reciprocal(
                        out=rinv[64:65, :], in_=otp[j][64:65, :])
                    rb = pa_rb.tile([64, NQL], F32, tag="rb")
                    nc.tensor.matmul(
                        rb,
                        lhsT=ones_col32[64:65, :],
                        rhs=rinv[64:65, :],
                        start=True, stop=True)
                    rb_sb = pa_ep.tile([64, NQL], F32, tag="rb_sb")
                    nc.scalar.copy(out=rb_sb, in_=rb)
                    if j == 0:
                        nc.vector.tensor_tensor(
                            out=ot_sb[t][0:64, :], in0=otp[j][0:64, :], in1=rb_sb,
                            op=mybir.AluOpType.mult)
                    else:
                        tmp = pa_ep.tile([64, NQL], F32R, tag="ottmp")
                        nc.vector.tensor_tensor(
                            out=tmp, in0=otp[j][0:64, :], in1=rb_sb,
                            op=mybir.AluOpType.mult)
                        # partition shift 0-63 -> 64-127 needs a DMA hop
                        nc.gpsimd.dma_start(out=ot_sb[t][64:128, :], in_=tmp)

        # ================= Phase Y: out proj + residual + LN =================
        with tc.tile_pool(name="py_w", bufs=1) as py_w, \
             tc.tile_pool(name="py_x", bufs=2) as py_x, \
             tc.tile_pool(name="py_t", bufs=2) as py_t, \
             tc.tile_pool(name="py_s", bufs=4) as py_s, \
             tc.tile_pool(name="py_ps", bufs=2, space="PSUM") as py_ps:
            wo_all = py_w.tile([P, HT, D], F32R, name="wo_all")
            nc.sync.dma_start(out=wo_all, in_=wo[:, :, :].bitcast(F32R))
            wo_sb = [wo_all[:, i, :] for i in range(HT)]
            for qt in range(QT_TILES):
                xres = py_x.tile([P, D], F32, tag="xres")
                nc.sync.dma_start(out=xres, in_=xq[qt * P:(qt + 1) * P, :])
                yps = py_ps.tile([P, D], F32, tag="yps")
                for half in range(2):
                    for ht in range(HT):
                        nc.tensor.matmul(
                            yps[:, half * 512:(half + 1) * 512],
                            lhsT=ot_sb[ht][:, qt * P:(qt + 1) * P],
                            rhs=wo_sb[ht][:, half * 512:(half + 1) * 512],
                            start=(ht == 0), stop=False)
                    nc.tensor.matmul(  # + bo (rank-1)
                        yps[:, half * 512:(half + 1) * 512],
                        lhsT=ones_col[0:1, 0:P],
                        rhs=bo_sb[0:1, half * 512:(half + 1) * 512],
                        start=False, stop=True)
                # residual add (psum + sbuf -> sbuf)
                x_t = py_t.tile([P, D], F32, tag="x_t")
                nc.vector.tensor_tensor(
                    out=x_t, in0=yps, in1=xres, op=mybir.AluOpType.add)
                # mean/var in one pass via bn_stats/bn_aggr
                nsub = D // nc.vector.BN_STATS_FMAX
                stats = py_s.tile([P, nsub, nc.vector.BN_STATS_DIM], F32,
                                  tag="stats")
                xg = x_t.rearrange("p (s f) -> p s f", s=nsub)
                for s in range(nsub):
                    nc.vector.bn_stats(out=stats[:, s, :], in_=xg[:, s, :])
                mv = py_s.tile([P, nc.vector.BN_AGGR_DIM], F32, tag="mv")
                nc.vector.bn_aggr(out=mv, in_=stats)
                var_eps = py_s.tile([P, 1], F32, tag="var_eps")
                nc.vector.tensor_scalar(
                    out=var_eps, in0=mv[:, 1:2], scalar1=LN_EPS, scalar2=None,
                    op0=mybir.AluOpType.add)
                rvar = py_s.tile([P, 1], F32, tag="rvar")
                nc.vector.reciprocal(out=rvar, in_=var_eps)
                rstd = py_s.tile([P, 1], F32, tag="rstd")
                nc.scalar.sqrt(out=rstd, in_=rvar)
                xhat = py_t.tile([P, D], F32, tag="xhat")
                nc.vector.tensor_scalar(
                    out=xhat, in0=x_t, scalar1=mv[:, 0:1], scalar2=rstd,
                    op0=mybir.AluOpType.subtract, op1=mybir.AluOpType.mult)
                yout = py_t.tile([P, D], F32, tag="yout")
                nc.vector.tensor_tensor(
                    out=yout, in0=xhat, in1=gamma_b, op=mybir.AluOpType.mult)
                nc.vector.tensor_tensor(
                    out=yout, in0=yout, in1=beta_b, op=mybir.AluOpType.add)
                nc.sync.dma_start(out=y[qt * P:(qt + 1) * P, :], in_=yout)

    nc.compile()
    return nc


_NC_CACHE = {}


def _get_nc():
    if "nc" not in _NC_CACHE:
        _NC_CACHE["nc"] = build_nc()
    return _NC_CACHE["nc"]


def make_in_maps(queries, keys, values, geometry, attention_mask,
                 Wq, bq, Wk, bk, Wv, bv, Wo, bo, ln_gamma, ln_beta):
    bf16 = mybir.dt.np(BF16)
    f32 = np.float32
    wq_s = (np.asarray(Wq, dtype=f32) * np.float32(QSCALE))
    bq_s = np.asarray(bq, dtype=f32) * np.float32(QSCALE)

    def pack_w(w):
        # [D, D] -> [P, DT_KT, D]: tile rows by 128, partition-major
        return np.ascontiguousarray(
            np.asarray(w, dtype=f32).reshape(DT_KT, P, D).transpose(1, 0, 2))

    shared = {
        "wq": pack_w(wq_s),
        "wk": pack_w(Wk),
        "wv": pack_w(Wv),
        "wo": pack_w(Wo),
        "bq_s": bq_s,
        "bk_in": np.ascontiguousarray(bk, dtype=f32),
        "bv_in": np.ascontiguousarray(bv, dtype=f32),
        "bo_in": np.ascontiguousarray(bo, dtype=f32),
        "ones_in": np.ones((P, D), dtype=f32),
        "ones_bf": np.ones((P, H), dtype=bf16),
        "gamma2d": np.broadcast_to(np.asarray(ln_gamma, dtype=f32), (P, D)).copy(),
        "beta2d": np.broadcast_to(np.asarray(ln_beta, dtype=f32), (P, D)).copy(),
    }

    # g_eff = where(mask, 0, clip(g, 1e-6)): fold the -inf mask into the
    # geometry gate (exactly equivalent post-softmax)
    g_eff = np.where(attention_mask, np.float32(0),
                     np.clip(geometry, 1e-6, None).astype(f32))  # [B,H,NQ,NK]

    def packT(x):  # [N, D] -> [DT_KT, P, cols] transposed contiguous tiles
        return np.ascontiguousarray(
            np.asarray(x, dtype=f32).T.reshape(DT_KT, P, -1))

    in_maps = []
    for c in range(NCORES):
        b, qh = c // 2, c % 2
        qs = slice(qh * NQL, (qh + 1) * NQL)
        # [H, NQL, NK] -> [H, NK, NQL] -> [H, KT, P, NQL]
        g_slice = g_eff[b, :, qs, :].transpose(0, 2, 1)
        in_maps.append({
            "xqT": packT(queries[b, qs]),
            "keysT": packT(keys[b]),
            "valuesT": packT(values[b]),
            "xq": np.ascontiguousarray(queries[b, qs], dtype=f32),
            "g_t": np.ascontiguousarray(
                g_slice.reshape(H, KT_TILES, P, NQL).astype(bf16)),
            **shared,
        })
    return in_maps


def kernel(queries, keys, values, geometry, attention_mask,
           Wq, bq, Wk, bk, Wv, bv, Wo, bo, ln_gamma, ln_beta, **run_kwargs):
    nc = _get_nc()
    in_maps = make_in_maps(queries, keys, values, geometry, attention_mask,
                           Wq, bq, Wk, bk, Wv, bv, Wo, bo, ln_gamma, ln_beta)
    res = run_bass_kernel_spmd(nc, in_maps, core_ids=list(range(NCORES)),
                               **run_kwargs)
    out = np.empty((B, NQ, D), np.float32)
    for c in range(NCORES):
        b, qh = c // 2, c % 2
        out[b, qh * NQL:(qh + 1) * NQL, :] = res.results[c]["y"]
    if run_kwargs:
        kernel.last_results = res
    return out


# revision 20
# speedup vs baseline: 6.4292x; 1.3291x over previous
"""Trainium2 Bass kernel for MultiHeadGeometryAttention.

Math (per batch b):
  q = x @ Wq + bq ; k = keys @ Wk + bk ; v = values @ Wv + bv   (per-head d=64)
  S_h = q_h k_h^T / 8
  w = softmax(log(clip(g,1e-6)) + where(mask, -inf, S))
    = g_eff * exp(S/8) / rowsum(...)    with g_eff = where(mask, 0, clip(g,1e-6))
  out = (w @ v) reshaped @ Wo + bo ; y = LayerNorm(x + out) * gamma + beta

Sharding: 8 cores = 4 batches x 2 query-halves (512 q rows per core).
Each core computes K/V projections for its batch (duplicated between the
2 cores of a pair) and everything else for its q rows. No collectives.

All DRAM inputs are pre-packed on the host so that every SBUF tile load is
a large contiguous block. Weights and projection inputs travel as bf16;
the residual path stays f32.

Device dataflow:
  QT' = (Wq/240)^T xqT  : [hd, nq]  bf16  (scale folded: exp scale=30)
  KT  = Wk^T keysT      : [hd, nk]  bf16
  Vaug= valuesT^T Wv    : [nk, 16*65] bf16 (per head: 64 v cols + ones col)
  ST'_h = KT_h^T QT'_h  : [nk, nq] in PSUM  (head pairs pack PE row tiles)
  pt  = exp(30*ST') * g_eff  (ACT exp straight from PSUM -> bf16, DVE mult)
  OT_h/r = (Vaug_h^T pt) : [65, nq], row 64 = rowsum r
  The attention loop is software-pipelined: S/exp/mult run AHEAD iterations
  ahead of the PV accumulations so the PE never head-of-line blocks.
  Y = OT^T Wo + bo ; + residual ; LayerNorm  (mean/var on DVE/ACT)
"""

import numpy as np
from contextlib import ExitStack

import concourse.bass as bass
import concourse.bacc as bacc
import concourse.tile as tile
from concourse import mybir
from concourse.bass_utils import run_bass_kernel_spmd

P = 128
B, NQ, NK, D, H, DK, DV = 4, 1024, 1024, 1024, 16, 64, 64
NQL = 512           # q rows per core
NCORES = 8
EXPS = 30.0         # exp scale; Wq pre-scaled by 1/(sqrt(64)*EXPS)
QSCALE = 1.0 / (8.0 * EXPS)
LN_EPS = 1e-5

F32 = mybir.dt.float32
F32R = mybir.dt.float32r
BF16 = mybir.dt.bfloat16

DT_KT = 8           # D // P contraction tiles
HT = H * DK // P    # 8 head-dim tiles of 128
KT_TILES = NK // P  # 8 key tiles
QT_TILES = NQL // P # 4 query tiles
AHEAD = 3           # attention software-pipeline depth (S ahead of PV)
EP_DELAY = 4        # iterations between last PV of a pair and its epilogue


def build_nc(with_bias=True, with_affine=True):
    nc = bacc.Bacc(None, target_bir_lowering=False)

    # host-packed layouts: every tile load below is contiguous in DRAM
    xqT = nc.dram_tensor("xqT", [DT_KT, P, NQL], BF16, kind="ExternalInput")
    keysT = nc.dram_tensor("keysT", [DT_KT, P, NK], BF16, kind="ExternalInput")
    valuesT = nc.dram_tensor("valuesT", [DT_KT, P, NK], BF16, kind="ExternalInput")
    xq = nc.dram_tensor("xq", [NQL, D], F32, kind="ExternalInput")
    # g pairs: [t][kt][p][j][q] so one DMA covers both heads of a pair
    g_t = nc.dram_tensor("g_t", [HT, KT_TILES, P, 2, NQL], BF16,
                         kind="ExternalInput")
    wq = nc.dram_tensor("wq", [P, DT_KT, D], BF16, kind="ExternalInput")
    wk = nc.dram_tensor("wk", [P, DT_KT, D], BF16, kind="ExternalInput")
    wv = nc.dram_tensor("wv", [P, DT_KT, D], BF16, kind="ExternalInput")
    wo = nc.dram_tensor("wo", [P, HT, D], BF16, kind="ExternalInput")
    ones_in = nc.dram_tensor("ones_in", [P, D], F32R, kind="ExternalInput")
    ones_bf = nc.dram_tensor("ones_bf", [P, H], BF16, kind="ExternalInput")
    e2_in = nc.dram_tensor("e2_in", [2, P], F32, kind="ExternalInput")
    gamma2d = nc.dram_tensor("gamma2d", [P, D], F32, kind="ExternalInput")
    beta2d = nc.dram_tensor("beta2d", [P, D], F32, kind="ExternalInput")
    if with_bias:
        bq = nc.dram_tensor("bq_s", [1, D], BF16, kind="ExternalInput")
        bk = nc.dram_tensor("bk_in", [1, D], BF16, kind="ExternalInput")
        bv = nc.dram_tensor("bv_in", [1, D], BF16, kind="ExternalInput")
        bo = nc.dram_tensor("bo_in", [1, D], BF16, kind="ExternalInput")
        ones_row_d = nc.dram_tensor("ones_row_d", [1, D], BF16,
                                    kind="ExternalInput")
    y = nc.dram_tensor("y", [NQL, D], F32, kind="ExternalOutput")

    with tile.TileContext(nc) as tc, ExitStack() as ctx:
        persist = ctx.enter_context(tc.tile_pool(name="persist", bufs=1))

        # ---- constants ----
        ones_hb = persist.tile([P, H], BF16, tag="ones_hb")
        nc.sync.dma_start(out=ones_hb, in_=ones_bf[:, :])
        e2_sb = persist.tile([2, P], F32, tag="e2_sb")
        nc.sync.dma_start(out=e2_sb, in_=e2_in[:, :])
        if with_bias:
            ones_rowb = persist.tile([1, D], BF16, tag="ones_rowb")
            nc.sync.dma_start(out=ones_rowb, in_=ones_row_d[0:1, :])
            bq_sb = persist.tile([1, D], BF16, tag="bq_sb")
            nc.sync.dma_start(out=bq_sb, in_=bq[0:1, :])
            bk_sb = persist.tile([1, D], BF16, tag="bk_sb")
            nc.sync.dma_start(out=bk_sb, in_=bk[0:1, :])
            bv_sb = persist.tile([1, D], BF16, tag="bv_sb")
            nc.sync.dma_start(out=bv_sb, in_=bv[0:1, :])
            bo_sb = persist.tile([1, D], BF16, tag="bo_sb")
            nc.sync.dma_start(out=bo_sb, in_=bo[0:1, :])
        gamma_b = persist.tile([P, D], F32, tag="gamma_b")
        nc.sync.dma_start(out=gamma_b, in_=gamma2d[:, :])
        beta_b = persist.tile([P, D], F32, tag="beta_b")
        nc.sync.dma_start(out=beta_b, in_=beta2d[:, :])

        # ---- persistent activations ----
        vaug = [persist.tile([P, H * 65], BF16, tag=f"vaug{i}", name=f"vaug{i}")
                for i in range(KT_TILES)]
        kt_sb = [persist.tile([P, NK], BF16, tag=f"kt{i}", name=f"kt{i}")
                 for i in range(HT)]
        qt_sb = [persist.tile([P, NQL], BF16, tag=f"qt{i}", name=f"qt{i}")
                 for i in range(HT)]
        ot_sb = [persist.tile([P, NQL], BF16, tag=f"ot{i}", name=f"ot{i}")
                 for i in range(HT)]

        def proj_group(ps, stat, mov, bias_lhsT, bias_rhs):
            """Accumulate sum_dt stat[dt]^T mov[dt] (+ rank-1 bias) into ps."""
            n = len(stat)
            for dt in range(n):
                last = (dt == n - 1) and not with_bias
                nc.tensor.matmul(ps, lhsT=stat[dt], rhs=mov[dt],
                                 start=(dt == 0), stop=last)
            if with_bias:
                nc.tensor.matmul(ps, lhsT=bias_lhsT, rhs=bias_rhs,
                                 start=False, stop=True)

        # ================= Phase V: Vaug = values^T-proj =================
        with tc.tile_pool(name="pv_in", bufs=1) as pv_in, \
             tc.tile_pool(name="pv_w", bufs=1) as pv_w, \
             tc.tile_pool(name="pv_ps", bufs=3, space="PSUM") as pv_ps:
            vT_all = pv_in.tile([P, DT_KT, NK], BF16, name="vT_all")
            wv_all = pv_w.tile([P, DT_KT, D], BF16, name="wv_all")
            for i in range(DT_KT):
                nc.sync.dma_start(out=vT_all[:, i, :], in_=valuesT[i])
            nc.sync.dma_start(out=wv_all, in_=wv[:, :, :])
            vT = [vT_all[:, i, :] for i in range(DT_KT)]
            wv_sb = [wv_all[:, i, :] for i in range(DT_KT)]
            for i in range(KT_TILES):
                nc.scalar.copy(
                    out=vaug[i].rearrange("p (h c) -> p h c", c=65)[:, :, 64:65],
                    in_=ones_hb[:, :, None])
                for half in range(2):
                    ps = pv_ps.tile([P, 512], F32, tag="vps")
                    proj_group(
                        ps,
                        [vT[dt][:, i * P:(i + 1) * P] for dt in range(DT_KT)],
                        [wv_sb[dt][:, half * 512:(half + 1) * 512]
                         for dt in range(DT_KT)],
                        None if not with_bias else ones_rowb[0:1, 0:P],
                        None if not with_bias else bv_sb[0:1, half * 512:(half + 1) * 512])
                    nc.scalar.copy(
                        out=vaug[i].rearrange("p (h c) -> p h c", c=65)
                            [:, half * 8:(half + 1) * 8, 0:64],
                        in_=ps.rearrange("p (h c) -> p h c", c=64))

        # ================= Phase K: KT = Wk^T keysT =================
        with tc.tile_pool(name="pk_in", bufs=1) as pk_in, \
             tc.tile_pool(name="pk_w", bufs=1) as pk_w, \
             tc.tile_pool(name="pk_ps", bufs=3, space="PSUM") as pk_ps:
            kT_all = pk_in.tile([P, DT_KT, NK], BF16, name="kT_all")
            wk_all = pk_w.tile([P, DT_KT, D], BF16, name="wk_all")
            for i in range(DT_KT):
                nc.sync.dma_start(out=kT_all[:, i, :], in_=keysT[i])
            nc.sync.dma_start(out=wk_all, in_=wk[:, :, :])
            kT = [kT_all[:, i, :] for i in range(DT_KT)]
            wk_sb = [wk_all[:, i, :] for i in range(DT_KT)]
            for ht in range(HT):
                for half in range(2):
                    ps = pk_ps.tile([P, 512], F32, tag="kps")
                    proj_group(
                        ps,
                        [wk_sb[dt][:, ht * P:(ht + 1) * P] for dt in range(DT_KT)],
                        [kT[dt][:, half * 512:(half + 1) * 512]
                         for dt in range(DT_KT)],
                        None if not with_bias else bk_sb[0:1, ht * P:(ht + 1) * P],
                        None if not with_bias else ones_rowb[0:1, 0:512])
                    nc.scalar.copy(
                        out=kt_sb[ht][:, half * 512:(half + 1) * 512], in_=ps)

        # ================= Phase Q: QT' = (Wq/240)^T xqT =================
        with tc.tile_pool(name="pq_in", bufs=1) as pq_in, \
             tc.tile_pool(name="pq_w", bufs=1) as pq_w, \
             tc.tile_pool(name="pq_ps", bufs=3, space="PSUM") as pq_ps:
            xqT_all = pq_in.tile([P, DT_KT, NQL], BF16, name="xqT_all")
            wq_all = pq_w.tile([P, DT_KT, D], BF16, name="wq_all")
            for i in range(DT_KT):
                nc.sync.dma_start(out=xqT_all[:, i, :], in_=xqT[i])
            nc.sync.dma_start(out=wq_all, in_=wq[:, :, :])
            xqT_sb = [xqT_all[:, i, :] for i in range(DT_KT)]
            wq_sb = [wq_all[:, i, :] for i in range(DT_KT)]
            for ht in range(HT):
                ps = pq_ps.tile([P, NQL], F32, tag="qps")
                proj_group(
                    ps,
                    [wq_sb[dt][:, ht * P:(ht + 1) * P] for dt in range(DT_KT)],
                    xqT_sb,
                    None if not with_bias else bq_sb[0:1, ht * P:(ht + 1) * P],
                    None if not with_bias else ones_rowb[0:1, 0:NQL])
                nc.scalar.copy(out=qt_sb[ht], in_=ps)

        # ================= Phase A: attention (software-pipelined) ========
        with tc.tile_pool(name="pa_gm", bufs=4) as pa_gm, \
             tc.tile_pool(name="pa_pt", bufs=AHEAD + 2) as pa_pt, \
             tc.tile_pool(name="pa_ep", bufs=2) as pa_ep, \
             tc.tile_pool(name="pa_st", bufs=3, space="PSUM") as pa_st, \
             tc.tile_pool(name="pa_ot", bufs=1, space="PSUM") as pa_ot, \
             tc.tile_pool(name="pa_rb", bufs=1, space="PSUM") as pa_rb:
            seq = [(t, kt, j) for t in range(HT)
                   for kt in range(KT_TILES) for j in range(2)]
            otp = {}
            gts = {}
            pts = {}
            pend_ep = []  # (t, emit_at_index)

            def emit_front(i):
                t, kt, j = seq[i]
                h = 2 * t + j
                if kt == 0 and j == 0:
                    otp[t] = [pa_ot.tile([65, NQL], F32, tag=f"otp{t % 2}_{jj}",
                                         name=f"otp{t}_{jj}")
                              for jj in range(2)]
                if j == 0:
                    gt = pa_gm.tile([P, 2, NQL], BF16, tag="gt")
                    nc.gpsimd.dma_start(out=gt, in_=g_t[t, kt])
                    gts[(t, kt)] = gt
                stp = pa_st.tile([P, NQL], F32, tag="stps")
                nc.tensor.matmul(
                    stp,
                    lhsT=kt_sb[t][64 * j:64 * j + 64, kt * P:(kt + 1) * P],
                    rhs=qt_sb[t][64 * j:64 * j + 64, :],
                    start=True, stop=True)
                pt0 = pa_pt.tile([P, NQL], BF16, tag="pt0")
                nc.scalar.activation(
                    out=pt0, in_=stp,
                    func=mybir.ActivationFunctionType.Exp, scale=EXPS)
                pt = pa_pt.tile([P, NQL], BF16, tag="pt")
                nc.vector.tensor_tensor(
                    out=pt, in0=pt0, in1=gts[(t, kt)][:, j, :],
                    op=mybir.AluOpType.mult)
                pts[(t, kt, j)] = pt

            def emit_pv(i):
                t, kt, j = seq[i]
                h = 2 * t + j
                nc.tensor.matmul(
                    otp[t][j],
                    lhsT=vaug[kt][:, h * 65:(h + 1) * 65],
                    rhs=pts.pop((t, kt, j)),
                    start=(kt == 0), stop=(kt == KT_TILES - 1))
                if kt == KT_TILES - 1 and j == 1:
                    pend_ep.append([t, i + EP_DELAY])

            def emit_epilogue(t):
                # gather the two rowsum rows, one batched reciprocal,
                # PE-broadcast 1/r to 64 rows per head, divide into ot_sb
                r2 = pa_ep.tile([2, NQL], F32, tag="r2")
                for j in range(2):
                    rs = pa_ep.tile([P, NQL], F32, tag=f"rs{j}")
                    nc.scalar.copy(out=rs[64:65, :], in_=otp[t][j][64:65, :])
                    nc.gpsimd.dma_start(out=r2[j:j + 1, :], in_=rs[64:65, :])
                rinv2 = pa_ep.tile([2, NQL], F32, tag="rinv2")
                nc.vector.reciprocal(out=rinv2, in_=r2)
                for j in range(2):
                    rb = pa_rb.tile([64, NQL], F32, tag="rb")
                    nc.tensor.matmul(
                        rb,
                        lhsT=e2_sb[:, j * 64:(j + 1) * 64],
                        rhs=rinv2,
                        start=True, stop=True)
                    rb_sb = pa_ep.tile([64, NQL], F32, tag="rb_sb")
                    nc.scalar.copy(out=rb_sb, in_=rb)
                    if j == 0:
                        nc.vector.tensor_tensor(
                            out=ot_sb[t][0:64, :], in0=otp[t][j][0:64, :],
                            in1=rb_sb, op=mybir.AluOpType.mult)
                    else:
                        tmp = pa_ep.tile([64, NQL], BF16, tag="ottmp")
                        nc.vector.tensor_tensor(
                            out=tmp, in0=otp[t][j][0:64, :], in1=rb_sb,
                            op=mybir.AluOpType.mult)
                        # partition shift 0-63 -> 64-127 needs a DMA hop
                        nc.gpsimd.dma_start(out=ot_sb[t][64:128, :], in_=tmp)

            n = len(seq)
            for i in range(n + AHEAD):
                if i < n:
                    emit_front(i)
                if i >= AHEAD:
                    emit_pv(i - AHEAD)
                while pend_ep and pend_ep[0][1] <= i:
                    emit_epilogue(pend_ep.pop(0)[0])
            while pend_ep:
                emit_epilogue(pend_ep.pop(0)[0])

        # ================= Phase Y: out proj + residual + LN =================
        with tc.tile_pool(name="py_w", bufs=1) as py_w, \
             tc.tile_pool(name="py_x", bufs=2) as py_x, \
             tc.tile_pool(name="py_t", bufs=2) as py_t, \
             tc.tile_pool(name="py_s", bufs=4) as py_s, \
             tc.tile_pool(name="py_ps", bufs=2, space="PSUM") as py_ps:
            wo_all = py_w.tile([P, HT, D], BF16, name="wo_all")
            nc.sync.dma_start(out=wo_all, in_=wo[:, :, :])
            wo_sb = [wo_all[:, i, :] for i in range(HT)]
            for qt in range(QT_TILES):
                xres = py_x.tile([P, D], F32, tag="xres")
                nc.sync.dma_start(out=xres, in_=xq[qt * P:(qt + 1) * P, :])
                yps = py_ps.tile([P, D], F32, tag="yps")
                for half in range(2):
                    for ht in range(HT):
                        last = (ht == HT - 1) and not with_bias
                        nc.tensor.matmul(
                            yps[:, half * 512:(half + 1) * 512],
                            lhsT=ot_sb[ht][:, qt * P:(qt + 1) * P],
                            rhs=wo_sb[ht][:, half * 512:(half + 1) * 512],
                            start=(ht == 0), stop=last)
                    if with_bias:
                        nc.tensor.matmul(
                            yps[:, half * 512:(half + 1) * 512],
                            lhsT=ones_rowb[0:1, 0:P],
                            rhs=bo_sb[0:1, half * 512:(half + 1) * 512],
                            start=False, stop=True)
                # residual add (psum + sbuf -> sbuf)
                x_t = py_t.tile([P, D], F32, tag="x_t")
                nc.vector.tensor_tensor(
                    out=x_t, in0=yps, in1=xres, op=mybir.AluOpType.add)
                # mean/var in one pass via bn_stats/bn_aggr
                nsub = D // nc.vector.BN_STATS_FMAX
                stats = py_s.tile([P, nsub, nc.vector.BN_STATS_DIM], F32,
                                  tag="stats")
                xg = x_t.rearrange("p (s f) -> p s f", s=nsub)
                for s in range(nsub):
                    nc.vector.bn_stats(out=stats[:, s, :], in_=xg[:, s, :])
                mv = py_s.tile([P, nc.vector.BN_AGGR_DIM], F32, tag="mv")
                nc.vector.bn_aggr(out=mv, in_=stats)
                var_eps = py_s.tile([P, 1], F32, tag="var_eps")
                nc.vector.tensor_scalar(
                    out=var_eps, in0=mv[:, 1:2], scalar1=LN_EPS, scalar2=None,
                    op0=mybir.AluOpType.add)
                rvar = py_s.tile([P, 1], F32, tag="rvar")
                nc.vector.reciprocal(out=rvar, in_=var_eps)
                rstd = py_s.tile([P, 1], F32, tag="rstd")
                nc.scalar.sqrt(out=rstd, in_=rvar)
                xhat = py_t.tile([P, D], F32, tag="xhat")
                nc.vector.tensor_scalar(
                    out=xhat, in0=x_t, scalar1=mv[:, 0:1], scalar2=rstd,
                    op0=mybir.AluOpType.subtract, op1=mybir.AluOpType.mult)
                if with_affine:
                    yout = py_t.tile([P, D], F32, tag="yout")
                    nc.vector.tensor_tensor(
                        out=yout, in0=xhat, in1=gamma_b, op=mybir.AluOpType.mult)
                    nc.vector.tensor_tensor(
                        out=yout, in0=yout, in1=beta_b, op=mybir.AluOpType.add)
                else:
                    yout = xhat
                nc.sync.dma_start(out=y[qt * P:(qt + 1) * P, :], in_=yout)

    nc.compile()
    return nc


_NC_CACHE = {}


def _get_nc(with_bias, with_affine):
    key = (with_bias, with_affine)
    if key not in _NC_CACHE:
        _NC_CACHE[key] = build_nc(with_bias, with_affine)
    return _NC_CACHE[key]


def make_in_maps(queries, keys, values, geometry, attention_mask,
                 Wq, bq, Wk, bk, Wv, bv, Wo, bo, ln_gamma, ln_beta,
                 with_bias, with_affine):
    bf16 = mybir.dt.np(BF16)
    f32 = np.float32
    wq_s = (np.asarray(Wq, dtype=f32) * np.float32(QSCALE))

    def pack_w(w):
        # [D, D] -> [P, DT_KT, D]: tile rows by 128, partition-major
        return np.ascontiguousarray(
            np.asarray(w, dtype=f32).reshape(DT_KT, P, D).transpose(1, 0, 2)
        ).astype(bf16)

    shared = {
        "wq": pack_w(wq_s),
        "wk": pack_w(Wk),
        "wv": pack_w(Wv),
        "wo": pack_w(Wo),
        "ones_in": np.ones((P, D), dtype=f32),
        "ones_bf": np.ones((P, H), dtype=bf16),
        "e2_in": np.kron(np.eye(2, dtype=f32), np.ones((1, 64), dtype=f32)),
        "gamma2d": np.broadcast_to(np.asarray(ln_gamma, dtype=f32), (P, D)).copy(),
        "beta2d": np.broadcast_to(np.asarray(ln_beta, dtype=f32), (P, D)).copy(),
    }
    if with_bias:
        bq_s = np.asarray(bq, dtype=f32) * np.float32(QSCALE)
        for nm, bb in (("bq_s", bq_s), ("bk_in", bk), ("bv_in", bv),
                       ("bo_in", bo)):
            shared[nm] = np.asarray(bb, dtype=f32).astype(bf16).reshape(1, D)
        shared["ones_row_d"] = np.ones((1, D), dtype=bf16)

    # g_eff = where(mask, 0, clip(g, 1e-6)): fold the -inf mask into the
    # geometry gate (exactly equivalent post-softmax)
    g_eff = np.where(attention_mask, np.float32(0),
                     np.clip(geometry, 1e-6, None).astype(f32))  # [B,H,NQ,NK]

    def packT(x):  # [N, D] -> [DT_KT, P, cols] transposed contiguous tiles
        return np.ascontiguousarray(
            np.asarray(x, dtype=f32).T.reshape(DT_KT, P, -1)).astype(bf16)

    in_maps = []
    for c in range(NCORES):
        b, qh = c // 2, c % 2
        qs = slice(qh * NQL, (qh + 1) * NQL)
        # [H, NQL, NK] -> [H, NK, NQL]; pair heads: [HT, KT, P, 2, NQL]
        g_slice = g_eff[b, :, qs, :].transpose(0, 2, 1).reshape(
            HT, 2, KT_TILES, P, NQL).transpose(0, 2, 3, 1, 4)
        in_maps.append({
            "xqT": packT(queries[b, qs]),
            "keysT": packT(keys[b]),
            "valuesT": packT(values[b]),
            "xq": np.ascontiguousarray(queries[b, qs], dtype=f32),
            "g_t": np.ascontiguousarray(g_slice.astype(bf16)),
            **shared,
        })
    return in_maps


def kernel(queries, keys, values, geometry, attention_mask,
           Wq, bq, Wk, bk, Wv, bv, Wo, bo, ln_gamma, ln_beta, **run_kwargs):
    with_bias = bool(np.any(bq) or np.any(bk) or np.any(bv) or np.any(bo))
    with_affine = bool(np.any(np.asarray(ln_gamma) != 1) or np.any(ln_beta))
    nc = _get_nc(with_bias, with_affine)
    in_maps = make_in_maps(queries, keys, values, geometry, attention_mask,
                           Wq, bq, Wk, bk, Wv, bv, Wo, bo, ln_gamma, ln_beta,
                           with_bias, with_affine)
    res = run_bass_kernel_spmd(nc, in_maps, core_ids=list(range(NCORES)),
                               **run_kwargs)
    out = np.empty((B, NQ, D), np.float32)
    for c in range(NCORES):
        b, qh = c // 2, c % 2
        out[b, qh * NQL:(qh + 1) * NQL, :] = res.results[c]["y"]
    if run_kwargs:
        kernel.last_results = res
    return out


# revision 26
# speedup vs baseline: 8.5541x; 1.3305x over previous
"""Trainium2 Bass kernel for MultiHeadGeometryAttention.

Math (per batch b):
  q = x @ Wq + bq ; k = keys @ Wk + bk ; v = values @ Wv + bv   (per-head d=64)
  S_h = q_h k_h^T / 8
  w = softmax(log(clip(g,1e-6)) + where(mask, -inf, S))
    = g_eff * exp(S/8 - c) / rowsum(...)   with g_eff = where(mask, 0, clip(g,1e-6))
  out = (w @ v) reshaped @ Wo + bo ; y = LayerNorm(x + out) * gamma + beta

Sharding: 8 cores = 4 batches x 2 query-halves (512 q rows per core).
No collectives; K/V projections duplicated within a batch pair.

Numerics: projections run in fp8(e4m3) DoubleRow mode (2 contraction tiles
per PE pass); attention scores stay bf16; softmax numerator exp(S/8-3) and
geometry gate are fp8 so the PV matmul also runs DoubleRow (2 key tiles per
pass).  The fixed -3 logit shift keeps exp() inside fp8 range and cancels
in the normalization.  Residual + LayerNorm stay f32.

All DRAM inputs are pre-packed on the host so every SBUF tile load is a
large contiguous block.
"""

import numpy as np
from contextlib import ExitStack

import concourse.bass as bass
import concourse.bacc as bacc
import concourse.tile as tile
from concourse import mybir
from concourse.bass_utils import run_bass_kernel_spmd

P = 128
B, NQ, NK, D, H, DK, DV = 4, 1024, 1024, 1024, 16, 64, 64
NQL = 512           # q rows per core
NCORES = 8
EXPS = 0.125        # exp scale: logits = S/8
EXPB = -3.0         # logit shift: keeps fp8 exp() in range; cancels in softmax
LN_EPS = 1e-5

F32 = mybir.dt.float32
F32R = mybir.dt.float32r
BF16 = mybir.dt.bfloat16
FP8 = mybir.dt.float8e4
DR = mybir.MatmulPerfMode.DoubleRow

DT_KT = 8           # D // P contraction tiles
HT = H * DK // P    # 8 head-dim tiles of 128
KT_TILES = NK // P  # 8 key tiles
MT = KT_TILES // 2  # 4 key-tile pairs
QT_TILES = NQL // P # 4 query tiles
AHEAD = 2           # attention software-pipeline depth (S/exp/mult ahead of PV)
EP_DELAY = 2        # iterations between a pair's last PV and its epilogue


def build_nc(with_bias=True, with_affine=True):
    nc = bacc.Bacc(None, target_bir_lowering=False)

    # host-packed layouts: every tile load below is contiguous in DRAM
    xqT = nc.dram_tensor("xqT", [DT_KT, P, NQL], FP8, kind="ExternalInput")
    keysT = nc.dram_tensor("keysT", [DT_KT, P, NK], FP8, kind="ExternalInput")
    valuesT = nc.dram_tensor("valuesT", [DT_KT, P, NK], FP8, kind="ExternalInput")
    xq = nc.dram_tensor("xq", [NQL, D], F32, kind="ExternalInput")
    # g gate: [t][j][p][m][slot][q]; one DMA per (t, j) covers 4 kt-pairs
    g_t = nc.dram_tensor("g_t", [HT, 2, P, MT, 2, NQL], FP8,
                         kind="ExternalInput")
    wq = nc.dram_tensor("wq", [P, DT_KT, D], FP8, kind="ExternalInput")
    wk = nc.dram_tensor("wk", [P, DT_KT, D], FP8, kind="ExternalInput")
    wv = nc.dram_tensor("wv", [P, DT_KT, D], FP8, kind="ExternalInput")
    wo = nc.dram_tensor("wo", [P, HT, D], BF16, kind="ExternalInput")
    ones_bf = nc.dram_tensor("ones_bf", [P, H], FP8, kind="ExternalInput")
    expb_in = nc.dram_tensor("expb_in", [P, 1], F32, kind="ExternalInput")
    gamma2d = nc.dram_tensor("gamma2d", [P, D], F32, kind="ExternalInput")
    beta2d = nc.dram_tensor("beta2d", [P, D], F32, kind="ExternalInput")
    if with_bias:
        bq = nc.dram_tensor("bq_s", [1, D], FP8, kind="ExternalInput")
        bk = nc.dram_tensor("bk_in", [1, D], FP8, kind="ExternalInput")
        bv = nc.dram_tensor("bv_in", [1, D], FP8, kind="ExternalInput")
        bo = nc.dram_tensor("bo_in", [1, D], BF16, kind="ExternalInput")
        ones_row_d = nc.dram_tensor("ones_row_d", [1, D], FP8,
                                    kind="ExternalInput")
        ones_row_b = nc.dram_tensor("ones_row_b", [1, D], BF16,
                                    kind="ExternalInput")
    y = nc.dram_tensor("y", [NQL, D], F32, kind="ExternalOutput")

    with tile.TileContext(nc) as tc, ExitStack() as ctx:
        persist = ctx.enter_context(tc.tile_pool(name="persist", bufs=1))

        # ---- constants ----
        ones_hb = persist.tile([P, H], FP8, tag="ones_hb")
        nc.sync.dma_start(out=ones_hb, in_=ones_bf[:, :])
        expb_sb = persist.tile([P, 1], F32, tag="expb_sb")
        nc.sync.dma_start(out=expb_sb, in_=expb_in[:, :])
        gamma_b = persist.tile([P, D], F32, tag="gamma_b")
        nc.sync.dma_start(out=gamma_b, in_=gamma2d[:, :])
        beta_b = persist.tile([P, D], F32, tag="beta_b")
        nc.sync.dma_start(out=beta_b, in_=beta2d[:, :])
        if with_bias:
            ones_rowf = persist.tile([1, D], FP8, tag="ones_rowf")
            nc.sync.dma_start(out=ones_rowf, in_=ones_row_d[0:1, :])
            ones_rowb = persist.tile([1, D], BF16, tag="ones_rowb")
            nc.sync.dma_start(out=ones_rowb, in_=ones_row_b[0:1, :])
            bq_sb = persist.tile([1, D], FP8, tag="bq_sb")
            nc.sync.dma_start(out=bq_sb, in_=bq[0:1, :])
            bk_sb = persist.tile([1, D], FP8, tag="bk_sb")
            nc.sync.dma_start(out=bk_sb, in_=bk[0:1, :])
            bv_sb = persist.tile([1, D], FP8, tag="bv_sb")
            nc.sync.dma_start(out=bv_sb, in_=bv[0:1, :])
            bo_sb = persist.tile([1, D], BF16, tag="bo_sb")
            nc.sync.dma_start(out=bo_sb, in_=bo[0:1, :])

        # ---- persistent activations ----
        # Vaug pairs: [m][128, 2, H*65] fp8 (per head: 64 v cols + ones col)
        vaug = [persist.tile([P, 2, H * 65], FP8, tag=f"vaug{m}",
                             name=f"vaug{m}") for m in range(MT)]
        kt_sb = [persist.tile([P, NK], BF16, tag=f"kt{i}", name=f"kt{i}")
                 for i in range(HT)]
        qt_sb = [persist.tile([P, NQL], BF16, tag=f"qt{i}", name=f"qt{i}")
                 for i in range(HT)]
        ot_sb = [persist.tile([P, NQL], BF16, tag=f"ot{i}", name=f"ot{i}")
                 for i in range(HT)]

        def proj_group(ps, stat_all, stat_cols, mov_all, mov_cols,
                       bias_lhsT, bias_rhs):
            """ps = sum_dt stat[:,dt,stat_cols]^T mov[:,dt,mov_cols] (+bias),
            fp8 DoubleRow over pairs of contraction tiles."""
            for dp in range(DT_KT // 2):
                last = (dp == DT_KT // 2 - 1) and not with_bias
                nc.tensor.matmul(
                    ps,
                    lhsT=stat_all[:, 2 * dp:2 * dp + 2, stat_cols],
                    rhs=mov_all[:, 2 * dp:2 * dp + 2, mov_cols],
                    start=(dp == 0), stop=last, perf_mode=DR)
            if with_bias:
                nc.tensor.matmul(ps, lhsT=bias_lhsT, rhs=bias_rhs,
                                 start=False, stop=True)

        # ================= Phase V: Vaug = values^T-proj =================
        with tc.tile_pool(name="pv_in", bufs=1) as pv_in, \
             tc.tile_pool(name="pv_w", bufs=1) as pv_w, \
             tc.tile_pool(name="pv_ps", bufs=3, space="PSUM") as pv_ps:
            vT_all = pv_in.tile([P, DT_KT, NK], FP8, name="vT_all")
            wv_all = pv_w.tile([P, DT_KT, D], FP8, name="wv_all")
            for i in range(DT_KT):
                nc.sync.dma_start(out=vT_all[:, i, :], in_=valuesT[i])
            nc.sync.dma_start(out=wv_all, in_=wv[:, :, :])
            for i in range(KT_TILES):
                m, slot = i // 2, i % 2
                nc.scalar.copy(
                    out=vaug[m][:, slot, :].rearrange(
                        "p (h c) -> p h c", c=65)[:, :, 64:65],
                    in_=ones_hb[:, :, None])
                for half in range(2):
                    ps = pv_ps.tile([P, 512], F32, tag="vps")
                    proj_group(
                        ps, vT_all, slice(i * P, (i + 1) * P),
                        wv_all, slice(half * 512, (half + 1) * 512),
                        None if not with_bias else ones_rowf[0:1, 0:P],
                        None if not with_bias else bv_sb[0:1, half * 512:(half + 1) * 512])
                    nc.scalar.copy(
                        out=vaug[m][:, slot, :].rearrange(
                            "p (h c) -> p h c", c=65)
                            [:, half * 8:(half + 1) * 8, 0:64],
                        in_=ps.rearrange("p (h c) -> p h c", c=64))

        # ================= Phase K: KT = Wk^T keysT =================
        with tc.tile_pool(name="pk_in", bufs=1) as pk_in, \
             tc.tile_pool(name="pk_w", bufs=1) as pk_w, \
             tc.tile_pool(name="pk_ps", bufs=3, space="PSUM") as pk_ps:
            kT_all = pk_in.tile([P, DT_KT, NK], FP8, name="kT_all")
            wk_all = pk_w.tile([P, DT_KT, D], FP8, name="wk_all")
            for i in range(DT_KT):
                nc.sync.dma_start(out=kT_all[:, i, :], in_=keysT[i])
            nc.sync.dma_start(out=wk_all, in_=wk[:, :, :])
            for ht in range(HT):
                for half in range(2):
                    ps = pk_ps.tile([P, 512], F32, tag="kps")
                    proj_group(
                        ps, wk_all, slice(ht * P, (ht + 1) * P),
                        kT_all, slice(half * 512, (half + 1) * 512),
                        None if not with_bias else bk_sb[0:1, ht * P:(ht + 1) * P],
                        None if not with_bias else ones_rowf[0:1, 0:512])
                    nc.scalar.copy(
                        out=kt_sb[ht][:, half * 512:(half + 1) * 512], in_=ps)

        # ================= Phase Q: QT = Wq^T xqT =================
        with tc.tile_pool(name="pq_in", bufs=1) as pq_in, \
             tc.tile_pool(name="pq_w", bufs=1) as pq_w, \
             tc.tile_pool(name="pq_ps", bufs=3, space="PSUM") as pq_ps:
            xqT_all = pq_in.tile([P, DT_KT, NQL], FP8, name="xqT_all")
            wq_all = pq_w.tile([P, DT_KT, D], FP8, name="wq_all")
            for i in range(DT_KT):
                nc.sync.dma_start(out=xqT_all[:, i, :], in_=xqT[i])
            nc.sync.dma_start(out=wq_all, in_=wq[:, :, :])
            for ht in range(HT):
                ps = pq_ps.tile([P, NQL], F32, tag="qps")
                proj_group(
                    ps, wq_all, slice(ht * P, (ht + 1) * P),
                    xqT_all, slice(0, NQL),
                    None if not with_bias else bq_sb[0:1, ht * P:(ht + 1) * P],
                    None if not with_bias else ones_rowf[0:1, 0:NQL])
                nc.scalar.copy(out=qt_sb[ht], in_=ps)

        # ========== Phase A: attention (software-pipelined, fp8 PV) ========
        with tc.tile_pool(name="pa_gm", bufs=3) as pa_gm, \
             tc.tile_pool(name="pa_pt", bufs=AHEAD + 2) as pa_pt, \
             tc.tile_pool(name="pa_ep", bufs=2) as pa_ep, \
             tc.tile_pool(name="pa_st", bufs=2, space="PSUM") as pa_st, \
             tc.tile_pool(name="pa_ot", bufs=1, space="PSUM") as pa_ot:
            seq = [(t, j, m) for t in range(HT) for j in range(2)
                   for m in range(MT)]
            otp = {}
            gts = {}
            pts = {}
            pend_ep = []  # [t, emit_at_index]

            def emit_front(i):
                t, j, m = seq[i]
                h = 2 * t + j
                if j == 0 and m == 0:
                    otp[t] = [pa_ot.tile([65, NQL], F32, tag=f"otp{t % 2}_{jj}",
                                         name=f"otp{t}_{jj}")
                              for jj in range(2)]
                if m == 0:
                    gt = pa_gm.tile([P, MT, 2, NQL], FP8, tag="gt")
                    nc.gpsimd.dma_start(out=gt, in_=g_t[t, j])
                    gts[(t, j)] = gt
                # two S matmuls (kt = 2m, 2m+1) into one 2-bank psum tile
                stp = pa_st.tile([P, 2, NQL], F32, tag="stps")
                for s in range(2):
                    kt = 2 * m + s
                    nc.tensor.matmul(
                        stp[:, s, :],
                        lhsT=kt_sb[t][64 * j:64 * j + 64, kt * P:(kt + 1) * P],
                        rhs=qt_sb[t][64 * j:64 * j + 64, :],
                        start=True, stop=True)
                pt0 = pa_pt.tile([P, 2, NQL], FP8, tag="pt0")
                nc.scalar.activation(
                    out=pt0, in_=stp,
                    func=mybir.ActivationFunctionType.Exp,
                    scale=EXPS, bias=expb_sb[:, 0:1])
                pt = pa_pt.tile([P, 2, NQL], FP8, tag="pt")
                nc.vector.tensor_tensor(
                    out=pt, in0=pt0, in1=gts[(t, j)][:, m, :, :],
                    op=mybir.AluOpType.mult)
                pts[(t, j, m)] = pt

            def emit_pv(i):
                t, j, m = seq[i]
                h = 2 * t + j
                nc.tensor.matmul(
                    otp[t][j],
                    lhsT=vaug[m][:, :, h * 65:(h + 1) * 65],
                    rhs=pts.pop((t, j, m)),
                    start=(m == 0), stop=(m == MT - 1), perf_mode=DR)
                if m == MT - 1 and j == 1:
                    pend_ep.append([t, i + EP_DELAY])

            def emit_epilogue(t):
                # gather both rowsum rows -> one reciprocal -> gpsimd
                # partition-broadcast -> divide into ot_sb
                r2 = pa_ep.tile([2, NQL], F32, tag="r2")
                for j in range(2):
                    rs = pa_ep.tile([P, NQL], F32, tag=f"rs{j}")
                    nc.scalar.copy(out=rs[64:65, :], in_=otp[t][j][64:65, :])
                    nc.gpsimd.dma_start(out=r2[j:j + 1, :], in_=rs[64:65, :])
                rinv2 = pa_ep.tile([2, NQL], F32, tag="rinv2")
                nc.vector.reciprocal(out=rinv2, in_=r2)
                rinv1 = pa_ep.tile([1, NQL], F32, tag="rinv1")
                nc.gpsimd.dma_start(out=rinv1, in_=rinv2[1:2, :])
                for j in range(2):
                    rb = pa_ep.tile([64, NQL], F32, tag=f"rb{j}")
                    nc.gpsimd.partition_broadcast(
                        out_ap=rb, in_ap=rinv2[0:1, :] if j == 0 else rinv1)
                    if j == 0:
                        nc.vector.tensor_tensor(
                            out=ot_sb[t][0:64, :], in0=otp[t][j][0:64, :],
                            in1=rb, op=mybir.AluOpType.mult)
                    else:
                        tmp = pa_ep.tile([64, NQL], BF16, tag="ottmp")
                        nc.vector.tensor_tensor(
                            out=tmp, in0=otp[t][j][0:64, :], in1=rb,
                            op=mybir.AluOpType.mult)
                        # partition shift 0-63 -> 64-127 needs a DMA hop
                        nc.gpsimd.dma_start(out=ot_sb[t][64:128, :], in_=tmp)

            n = len(seq)
            for i in range(n + AHEAD):
                if i < n:
                    emit_front(i)
                if i >= AHEAD:
                    emit_pv(i - AHEAD)
                while pend_ep and pend_ep[0][1] <= i:
                    emit_epilogue(pend_ep.pop(0)[0])
            while pend_ep:
                emit_epilogue(pend_ep.pop(0)[0])

        # ================= Phase Y: out proj + residual + LN =================
        with tc.tile_pool(name="py_w", bufs=1) as py_w, \
             tc.tile_pool(name="py_x", bufs=2) as py_x, \
             tc.tile_pool(name="py_t", bufs=2) as py_t, \
             tc.tile_pool(name="py_s", bufs=4) as py_s, \
             tc.tile_pool(name="py_ps", bufs=2, space="PSUM") as py_ps:
            wo_all = py_w.tile([P, HT, D], BF16, name="wo_all")
            nc.sync.dma_start(out=wo_all, in_=wo[:, :, :])
            wo_sb = [wo_all[:, i, :] for i in range(HT)]
            for qt in range(QT_TILES):
                xres = py_x.tile([P, D], F32, tag="xres")
                nc.sync.dma_start(out=xres, in_=xq[qt * P:(qt + 1) * P, :])
                yps = py_ps.tile([P, D], F32, tag="yps")
                for half in range(2):
                    for ht in range(HT):
                        last = (ht == HT - 1) and not with_bias
                        nc.tensor.matmul(
                            yps[:, half * 512:(half + 1) * 512],
                            lhsT=ot_sb[ht][:, qt * P:(qt + 1) * P],
                            rhs=wo_sb[ht][:, half * 512:(half + 1) * 512],
                            start=(ht == 0), stop=last)
                    if with_bias:
                        nc.tensor.matmul(
                            yps[:, half * 512:(half + 1) * 512],
                            lhsT=ones_rowb[0:1, 0:P],
                            rhs=bo_sb[0:1, half * 512:(half + 1) * 512],
                            start=False, stop=True)
                # residual add (psum + sbuf -> sbuf)
                x_t = py_t.tile([P, D], F32, tag="x_t")
                nc.vector.tensor_tensor(
                    out=x_t, in0=yps, in1=xres, op=mybir.AluOpType.add)
                # mean/var in one pass via bn_stats/bn_aggr
                nsub = D // nc.vector.BN_STATS_FMAX
                stats = py_s.tile([P, nsub, nc.vector.BN_STATS_DIM], F32,
                                  tag="stats")
                xg = x_t.rearrange("p (s f) -> p s f", s=nsub)
                for s in range(nsub):
                    nc.vector.bn_stats(out=stats[:, s, :], in_=xg[:, s, :])
                mv = py_s.tile([P, nc.vector.BN_AGGR_DIM], F32, tag="mv")
                nc.vector.bn_aggr(out=mv, in_=stats)
                var_eps = py_s.tile([P, 1], F32, tag="var_eps")
                nc.vector.tensor_scalar(
                    out=var_eps, in0=mv[:, 1:2], scalar1=LN_EPS, scalar2=None,
                    op0=mybir.AluOpType.add)
                rvar = py_s.tile([P, 1], F32, tag="rvar")
                nc.vector.reciprocal(out=rvar, in_=var_eps)
                rstd = py_s.tile([P, 1], F32, tag="rstd")
                nc.scalar.sqrt(out=rstd, in_=rvar)
                xhat = py_t.tile([P, D], F32, tag="xhat")
                nc.vector.tensor_scalar(
                    out=xhat, in0=x_t, scalar1=mv[:, 0:1], scalar2=rstd,
                    op0=mybir.AluOpType.subtract, op1=mybir.AluOpType.mult)
                if with_affine:
                    yout = py_t.tile([P, D], F32, tag="yout")
                    nc.vector.tensor_tensor(
                        out=yout, in0=xhat, in1=gamma_b, op=mybir.AluOpType.mult)
                    nc.vector.tensor_tensor(
                        out=yout, in0=yout, in1=beta_b, op=mybir.AluOpType.add)
                else:
                    yout = xhat
                nc.sync.dma_start(out=y[qt * P:(qt + 1) * P, :], in_=yout)

    nc.compile()
    return nc


_NC_CACHE = {}


def _get_nc(with_bias, with_affine):
    key = (with_bias, with_affine)
    if key not in _NC_CACHE:
        _NC_CACHE[key] = build_nc(with_bias, with_affine)
    return _NC_CACHE[key]


def make_in_maps(queries, keys, values, geometry, attention_mask,
                 Wq, bq, Wk, bk, Wv, bv, Wo, bo, ln_gamma, ln_beta,
                 with_bias, with_affine):
    bf16 = mybir.dt.np(BF16)
    fp8 = mybir.dt.np(FP8)
    f32 = np.float32

    def pack_w(w, dt):
        # [D, D] -> [P, DT_KT, D]: tile rows by 128, partition-major
        return np.ascontiguousarray(
            np.asarray(w, dtype=f32).reshape(DT_KT, P, D).transpose(1, 0, 2)
        ).astype(dt)

    shared = {
        "wq": pack_w(Wq, fp8),
        "wk": pack_w(Wk, fp8),
        "wv": pack_w(Wv, fp8),
        "wo": pack_w(Wo, bf16),
        "ones_bf": np.ones((P, H), dtype=fp8),
        "expb_in": np.full((P, 1), EXPB, dtype=f32),
        "gamma2d": np.broadcast_to(np.asarray(ln_gamma, dtype=f32), (P, D)).copy(),
        "beta2d": np.broadcast_to(np.asarray(ln_beta, dtype=f32), (P, D)).copy(),
    }
    if with_bias:
        for nm, bb, dt in (("bq_s", bq, fp8), ("bk_in", bk, fp8),
                           ("bv_in", bv, fp8), ("bo_in", bo, bf16)):
            shared[nm] = np.asarray(bb, dtype=f32).astype(dt).reshape(1, D)
        shared["ones_row_d"] = np.ones((1, D), dtype=fp8)
        shared["ones_row_b"] = np.ones((1, D), dtype=bf16)

    # g_eff = where(mask, 0, clip(g, 1e-6)): fold the -inf mask into the
    # geometry gate (exactly equivalent post-softmax)
    g_eff = np.where(attention_mask, np.float32(0),
                     np.clip(geometry, 1e-6, None).astype(f32))  # [B,H,NQ,NK]

    def packT(x):  # [N, D] -> [DT_KT, P, cols] transposed contiguous tiles
        return np.ascontiguousarray(
            np.asarray(x, dtype=f32).T.reshape(DT_KT, P, -1)).astype(fp8)

    in_maps = []
    for c in range(NCORES):
        b, qh = c // 2, c % 2
        qs = slice(qh * NQL, (qh + 1) * NQL)
        # [H, NQL, NK] -> [H=2t+j, NK=256m+128s+p, NQL] -> [t,j,p,m,s,q]
        g_slice = g_eff[b, :, qs, :].transpose(0, 2, 1).reshape(
            HT, 2, MT, 2, P, NQL).transpose(0, 1, 4, 2, 3, 5)
        in_maps.append({
            "xqT": packT(queries[b, qs]),
            "keysT": packT(keys[b]),
            "valuesT": packT(values[b]),
            "xq": np.ascontiguousarray(queries[b, qs], dtype=f32),
            "g_t": np.ascontiguousarray(g_slice.astype(fp8)),
            **shared,
        })
    return in_maps


def kernel(queries, keys, values, geometry, attention_mask,
           Wq, bq, Wk, bk, Wv, bv, Wo, bo, ln_gamma, ln_beta, **run_kwargs):
    with_bias = bool(np.any(bq) or np.any(bk) or np.any(bv) or np.any(bo))
    with_affine = bool(np.any(np.asarray(ln_gamma) != 1) or np.any(ln_beta))
    nc = _get_nc(with_bias, with_affine)
    in_maps = make_in_maps(queries, keys, values, geometry, attention_mask,
                           Wq, bq, Wk, bk, Wv, bv, Wo, bo, ln_gamma, ln_beta,
                           with_bias, with_affine)
    res = run_bass_kernel_spmd(nc, in_maps, core_ids=list(range(NCORES)),
                               **run_kwargs)
    out = np.empty((B, NQ, D), np.float32)
    for c in range(NCORES):
        b, qh = c // 2, c % 2
        out[b, qh * NQL:(qh + 1) * NQL, :] = res.results[c]["y"]
    if run_kwargs:
        kernel.last_results = res
    return out


# revision 31
# speedup vs baseline: 8.8897x; 1.0392x over previous
"""Trainium2 Bass kernel for MultiHeadGeometryAttention.

Math (per batch b):
  q = x @ Wq + bq ; k = keys @ Wk + bk ; v = values @ Wv + bv   (per-head d=64)
  S_h = q_h k_h^T / 8
  w = softmax(log(clip(g,1e-6)) + where(mask, -inf, S))
    = g_eff * exp(S/8 - c) / rowsum(...)   with g_eff = where(mask, 0, clip(g,1e-6))
  out = (w @ v) reshaped @ Wo + bo ; y = LayerNorm(x + out) * gamma + beta

Sharding: 8 cores = 4 batches x 2 query-halves (512 q rows per core).
No collectives; K/V projections duplicated within a batch pair.

Numerics: every matmul except the score PSUM accumulation and the output
projection runs in fp8(e4m3) DoubleRow mode (two contraction tiles per PE
pass).  For the scores the d=64 contraction is packed as [32 partitions x
2 slots]; the K/Q projection weights are column-permuted on the host so
the projections write that layout directly.  exp(S/8-3) and the geometry
gate are fp8 so the PV matmul is DoubleRow over key-tile pairs; the -3
logit shift keeps exp() inside fp8 range and cancels in the softmax
normalization.  The output projection runs bf16; residual + LayerNorm f32.

All DRAM inputs are pre-packed on the host so every SBUF tile load is a
large contiguous block; input loads are hoisted so DMA runs ahead of
compute.  The attention loop is software-pipelined (S/exp/gate run AHEAD
iterations ahead of the PV accumulation so the PE never head-of-line
blocks).
"""

import numpy as np
from contextlib import ExitStack

import concourse.bass as bass
import concourse.bacc as bacc
import concourse.tile as tile
from concourse import mybir
from concourse.bass_utils import run_bass_kernel_spmd

P = 128
B, NQ, NK, D, H, DK, DV = 4, 1024, 1024, 1024, 16, 64, 64
NQL = 512           # q rows per core
NCORES = 8
EXPS = 0.125        # exp scale: logits = S/8
EXPB = -3.0         # logit shift: keeps fp8 exp() in range; cancels in softmax
LN_EPS = 1e-5

F32 = mybir.dt.float32
BF16 = mybir.dt.bfloat16
FP8 = mybir.dt.float8e4
DR = mybir.MatmulPerfMode.DoubleRow

DT_KT = 8           # D // P contraction tiles
HT = H * DK // P    # 8 head-dim tiles of 128
G4 = H // 4         # 4 groups of 4 heads (for the packed score layout)
KT_TILES = NK // P  # 8 key tiles
MT = KT_TILES // 2  # 4 key-tile pairs
QT_TILES = NQL // P # 4 query tiles
AHEAD = 2           # attention software-pipeline depth
EP_DELAY = 2        # iterations between a pair's last PV and its epilogue


def build_nc(with_bias=True, with_affine=True):
    nc = bacc.Bacc(None, target_bir_lowering=False)

    # host-packed layouts: every tile load below is contiguous in DRAM
    xqT = nc.dram_tensor("xqT", [DT_KT, P, NQL], FP8, kind="ExternalInput")
    keysT = nc.dram_tensor("keysT", [DT_KT, P, NK], FP8, kind="ExternalInput")
    valuesT = nc.dram_tensor("valuesT", [DT_KT, P, NK], FP8, kind="ExternalInput")
    xq = nc.dram_tensor("xq", [NQL, D], F32, kind="ExternalInput")
    # g gate: [t][j][p][m][slot][q]; one DMA per (t, j) covers 4 kt-pairs
    g_t = nc.dram_tensor("g_t", [HT, 2, P, MT, 2, NQL], FP8,
                         kind="ExternalInput")
    wq = nc.dram_tensor("wq", [P, DT_KT, D], FP8, kind="ExternalInput")
    wk = nc.dram_tensor("wk", [P, DT_KT, D], FP8, kind="ExternalInput")
    wv = nc.dram_tensor("wv", [P, DT_KT, D], FP8, kind="ExternalInput")
    wo = nc.dram_tensor("wo", [P, HT, D], BF16, kind="ExternalInput")
    ones_bf = nc.dram_tensor("ones_bf", [P, H], FP8, kind="ExternalInput")
    expb_in = nc.dram_tensor("expb_in", [P, 1], F32, kind="ExternalInput")
    gamma2d = nc.dram_tensor("gamma2d", [P, D], F32, kind="ExternalInput")
    beta2d = nc.dram_tensor("beta2d", [P, D], F32, kind="ExternalInput")
    if with_bias:
        bq = nc.dram_tensor("bq_s", [1, D], FP8, kind="ExternalInput")
        bk = nc.dram_tensor("bk_in", [1, D], FP8, kind="ExternalInput")
        bv = nc.dram_tensor("bv_in", [1, D], FP8, kind="ExternalInput")
        bo = nc.dram_tensor("bo_in", [1, D], BF16, kind="ExternalInput")
        ones_row_d = nc.dram_tensor("ones_row_d", [1, D], FP8,
                                    kind="ExternalInput")
        ones_row_b = nc.dram_tensor("ones_row_b", [1, D], BF16,
                                    kind="ExternalInput")
    y = nc.dram_tensor("y", [NQL, D], F32, kind="ExternalOutput")

    with tile.TileContext(nc) as tc, ExitStack() as ctx:
        persist = ctx.enter_context(tc.tile_pool(name="persist", bufs=1))

        # ---- constants + all input loads, hoisted so DMA runs early ----
        ones_hb = persist.tile([P, H], FP8, tag="ones_hb")
        nc.sync.dma_start(out=ones_hb, in_=ones_bf[:, :])
        expb_sb = persist.tile([P, 1], F32, tag="expb_sb")
        nc.sync.dma_start(out=expb_sb, in_=expb_in[:, :])
        gamma_b = persist.tile([P, D], F32, tag="gamma_b")
        nc.sync.dma_start(out=gamma_b, in_=gamma2d[:, :])
        beta_b = persist.tile([P, D], F32, tag="beta_b")
        nc.sync.dma_start(out=beta_b, in_=beta2d[:, :])
        if with_bias:
            ones_rowf = persist.tile([1, D], FP8, tag="ones_rowf")
            nc.sync.dma_start(out=ones_rowf, in_=ones_row_d[0:1, :])
            ones_rowb = persist.tile([1, D], BF16, tag="ones_rowb")
            nc.sync.dma_start(out=ones_rowb, in_=ones_row_b[0:1, :])
            bq_sb = persist.tile([1, D], FP8, tag="bq_sb")
            nc.sync.dma_start(out=bq_sb, in_=bq[0:1, :])
            bk_sb = persist.tile([1, D], FP8, tag="bk_sb")
            nc.sync.dma_start(out=bk_sb, in_=bk[0:1, :])
            bv_sb = persist.tile([1, D], FP8, tag="bv_sb")
            nc.sync.dma_start(out=bv_sb, in_=bv[0:1, :])
            bo_sb = persist.tile([1, D], BF16, tag="bo_sb")
            nc.sync.dma_start(out=bo_sb, in_=bo[0:1, :])

        vT_all = persist.tile([P, DT_KT, NK], FP8, name="vT_all")
        wv_all = persist.tile([P, DT_KT, D], FP8, name="wv_all")
        kT_all = persist.tile([P, DT_KT, NK], FP8, name="kT_all")
        wk_all = persist.tile([P, DT_KT, D], FP8, name="wk_all")
        xqT_all = persist.tile([P, DT_KT, NQL], FP8, name="xqT_all")
        wq_all = persist.tile([P, DT_KT, D], FP8, name="wq_all")
        wo_all = persist.tile([P, HT, D], BF16, name="wo_all")
        for i in range(DT_KT):
            nc.sync.dma_start(out=vT_all[:, i, :], in_=valuesT[i])
        nc.sync.dma_start(out=wv_all, in_=wv[:, :, :])
        for i in range(DT_KT):
            nc.sync.dma_start(out=kT_all[:, i, :], in_=keysT[i])
        nc.sync.dma_start(out=wk_all, in_=wk[:, :, :])
        for i in range(DT_KT):
            nc.scalar.dma_start(out=xqT_all[:, i, :], in_=xqT[i])
        nc.scalar.dma_start(out=wq_all, in_=wq[:, :, :])
        nc.scalar.dma_start(out=wo_all, in_=wo[:, :, :])
        wo_sb = [wo_all[:, i, :] for i in range(HT)]

        # ---- persistent activations ----
        # Vaug pairs: [m][128, 2, H*65] fp8 (per head: 64 v cols + ones col)
        vaug = [persist.tile([P, 2, H * 65], FP8, tag=f"vaug{m}",
                             name=f"vaug{m}") for m in range(MT)]
        # packed scores layout: per 4-head group g, [32a:32a+32) holds head
        # 4g+a with d = 32*slot + (p-32a)
        kt8 = [persist.tile([P, 2, NK], FP8, tag=f"kt8{g}", name=f"kt8{g}")
               for g in range(G4)]
        qt8 = [persist.tile([P, 2, NQL], FP8, tag=f"qt8{g}", name=f"qt8{g}")
               for g in range(G4)]
        # base partition 96 is not addressable by engines: head a=3 of each
        # group lives in a hopped-down copy at base 0
        ktb = [persist.tile([32, 2, NK], FP8, tag=f"ktb{g}", name=f"ktb{g}")
               for g in range(G4)]
        qtb = [persist.tile([32, 2, NQL], FP8, tag=f"qtb{g}", name=f"qtb{g}")
               for g in range(G4)]
        ot_sb = [persist.tile([P, NQL], BF16, tag=f"ot{i}", name=f"ot{i}")
                 for i in range(HT)]

        def proj_group(ps, stat_all, stat_cols, mov_all, mov_cols,
                       bias_lhsT, bias_rhs):
            """ps = sum_dt stat[:,dt,stat_cols]^T mov[:,dt,mov_cols] (+bias),
            fp8 DoubleRow over pairs of contraction tiles."""
            for dp in range(DT_KT // 2):
                last = (dp == DT_KT // 2 - 1) and not with_bias
                nc.tensor.matmul(
                    ps,
                    lhsT=stat_all[:, 2 * dp:2 * dp + 2, stat_cols],
                    rhs=mov_all[:, 2 * dp:2 * dp + 2, mov_cols],
                    start=(dp == 0), stop=last, perf_mode=DR)
            if with_bias:
                nc.tensor.matmul(ps, lhsT=bias_lhsT, rhs=bias_rhs,
                                 start=False, stop=True)

        # ================= Phase V: Vaug = values^T-proj =================
        with tc.tile_pool(name="pv_ps", bufs=2, space="PSUM") as pv_ps:
            for i in range(KT_TILES):
                m, slot = i // 2, i % 2
                nc.scalar.copy(
                    out=vaug[m][:, slot, :].rearrange(
                        "p (h c) -> p h c", c=65)[:, :, 64:65],
                    in_=ones_hb[:, :, None])
                ps = pv_ps.tile([P, 2, 512], F32, tag="vps")
                for half in range(2):
                    proj_group(
                        ps[:, half, :], vT_all, slice(i * P, (i + 1) * P),
                        wv_all, slice(half * 512, (half + 1) * 512),
                        None if not with_bias else ones_rowf[0:1, 0:P],
                        None if not with_bias else bv_sb[0:1, half * 512:(half + 1) * 512])
                nc.scalar.copy(
                    out=vaug[m][:, slot, :].rearrange(
                        "p (h c) -> p h c", c=65)[:, :, 0:64],
                    in_=ps.rearrange("p a (h c) -> p (a h) c", c=64))

        # ======== Phase K/Q: packed-layout projections (wk/wq are host
        # column-permuted so out partition p, group g, slot s = head
        # 4g+p//32, d = 32s+p%32) ========
        with tc.tile_pool(name="pk_ps", bufs=2, space="PSUM") as pk_ps:
            for g in range(G4):
                for s in range(2):
                    c0 = (2 * g + s) * P
                    ps = pk_ps.tile([P, 2, 512], F32, tag="kps")
                    for half in range(2):
                        proj_group(
                            ps[:, half, :], wk_all, slice(c0, c0 + P),
                            kT_all, slice(half * 512, (half + 1) * 512),
                            None if not with_bias else bk_sb[0:1, c0:c0 + P],
                            None if not with_bias else ones_rowf[0:1, 0:512])
                    nc.scalar.copy(
                        out=kt8[g][:, s, :],
                        in_=ps.rearrange("p a b -> p (a b)"))
                nc.gpsimd.dma_start(out=ktb[g], in_=kt8[g][96:128, :, :])
        with tc.tile_pool(name="pq_ps", bufs=2, space="PSUM") as pq_ps:
            for g in range(G4):
                for s in range(2):
                    c0 = (2 * g + s) * P
                    ps = pq_ps.tile([P, NQL], F32, tag="qps")
                    proj_group(
                        ps, wq_all, slice(c0, c0 + P),
                        xqT_all, slice(0, NQL),
                        None if not with_bias else bq_sb[0:1, c0:c0 + P],
                        None if not with_bias else ones_rowf[0:1, 0:NQL])
                    nc.scalar.copy(out=qt8[g][:, s, :], in_=ps)
                nc.gpsimd.dma_start(out=qtb[g], in_=qt8[g][96:128, :, :])

        # ========== Phase A: attention (software-pipelined, all-fp8) ========
        with tc.tile_pool(name="pa_gm", bufs=3) as pa_gm, \
             tc.tile_pool(name="pa_pt", bufs=AHEAD + 2) as pa_pt, \
             tc.tile_pool(name="pa_ep", bufs=2) as pa_ep, \
             tc.tile_pool(name="pa_st", bufs=2, space="PSUM") as pa_st, \
             tc.tile_pool(name="pa_ot", bufs=1, space="PSUM") as pa_ot:
            seq = [(t, j, m) for t in range(HT) for j in range(2)
                   for m in range(MT)]
            otp = {}
            gts = {}
            pts = {}
            pend_ep = []  # [t, emit_at_index]

            def emit_front(i):
                t, j, m = seq[i]
                h = 2 * t + j
                g, a = h // 4, h % 4
                if j == 0 and m == 0:
                    otp[t] = [pa_ot.tile([65, NQL], F32, tag=f"otp{t % 2}_{jj}",
                                         name=f"otp{t}_{jj}")
                              for jj in range(2)]
                if m == 0:
                    gt = pa_gm.tile([P, MT, 2, NQL], FP8, tag="gt")
                    nc.gpsimd.dma_start(out=gt, in_=g_t[t, j])
                    gts[(t, j)] = gt
                # two DoubleRow S matmuls (kt = 2m, 2m+1), packed d=[32x2]
                stp = pa_st.tile([P, 2, NQL], F32, tag="stps")
                for s in range(2):
                    kt = 2 * m + s
                    if a == 3:
                        kl = ktb[g][:, :, kt * P:(kt + 1) * P]
                        qr = qtb[g][:, :, :]
                    else:
                        kl = kt8[g][32 * a:32 * a + 32, :, kt * P:(kt + 1) * P]
                        qr = qt8[g][32 * a:32 * a + 32, :, :]
                    nc.tensor.matmul(
                        stp[:, s, :], lhsT=kl, rhs=qr,
                        start=True, stop=True, perf_mode=DR)
                pt0 = pa_pt.tile([P, 2, NQL], FP8, tag="pt0")
                nc.scalar.activation(
                    out=pt0, in_=stp,
                    func=mybir.ActivationFunctionType.Exp,
                    scale=EXPS, bias=expb_sb[:, 0:1])
                pt = pa_pt.tile([P, 2, NQL], FP8, tag="pt")
                nc.vector.tensor_tensor(
                    out=pt, in0=pt0, in1=gts[(t, j)][:, m, :, :],
                    op=mybir.AluOpType.mult)
                pts[(t, j, m)] = pt

            def emit_pv(i):
                t, j, m = seq[i]
                h = 2 * t + j
                nc.tensor.matmul(
                    otp[t][j],
                    lhsT=vaug[m][:, :, h * 65:(h + 1) * 65],
                    rhs=pts.pop((t, j, m)),
                    start=(m == 0), stop=(m == MT - 1), perf_mode=DR)
                if m == MT - 1 and j == 1:
                    pend_ep.append([t, i + EP_DELAY])

            def emit_epilogue(t):
                # gather both rowsum rows -> one reciprocal -> gpsimd
                # partition-broadcast -> divide into ot_sb
                r2 = pa_ep.tile([2, NQL], F32, tag="r2")
                for j in range(2):
                    rs = pa_ep.tile([P, NQL], F32, tag=f"rs{j}")
                    nc.scalar.copy(out=rs[64:65, :], in_=otp[t][j][64:65, :])
                    nc.gpsimd.dma_start(out=r2[j:j + 1, :], in_=rs[64:65, :])
                rinv2 = pa_ep.tile([2, NQL], F32, tag="rinv2")
                nc.vector.reciprocal(out=rinv2, in_=r2)
                rinv1 = pa_ep.tile([1, NQL], F32, tag="rinv1")
                nc.gpsimd.dma_start(out=rinv1, in_=rinv2[1:2, :])
                for j in range(2):
                    rb = pa_ep.tile([64, NQL], F32, tag=f"rb{j}")
                    nc.gpsimd.partition_broadcast(
                        out_ap=rb, in_ap=rinv2[0:1, :] if j == 0 else rinv1)
                    if j == 0:
                        nc.vector.tensor_tensor(
                            out=ot_sb[t][0:64, :], in0=otp[t][j][0:64, :],
                            in1=rb, op=mybir.AluOpType.mult)
                    else:
                        tmp = pa_ep.tile([64, NQL], BF16, tag="ottmp")
                        nc.vector.tensor_tensor(
                            out=tmp, in0=otp[t][j][0:64, :], in1=rb,
                            op=mybir.AluOpType.mult)
                        # partition shift 0-63 -> 64-127 needs a DMA hop
                        nc.gpsimd.dma_start(out=ot_sb[t][64:128, :], in_=tmp)

            n = len(seq)
            for i in range(n + AHEAD):
                if i < n:
                    emit_front(i)
                if i >= AHEAD:
                    emit_pv(i - AHEAD)
                while pend_ep and pend_ep[0][1] <= i:
                    emit_epilogue(pend_ep.pop(0)[0])
            while pend_ep:
                emit_epilogue(pend_ep.pop(0)[0])

        # ================= Phase Y: out proj + residual + LN =================
        with tc.tile_pool(name="py_x", bufs=2) as py_x, \
             tc.tile_pool(name="py_t", bufs=2) as py_t, \
             tc.tile_pool(name="py_s", bufs=4) as py_s, \
             tc.tile_pool(name="py_ps", bufs=2, space="PSUM") as py_ps:
            for qt in range(QT_TILES):
                xres = py_x.tile([P, D], F32, tag="xres")
                nc.sync.dma_start(out=xres, in_=xq[qt * P:(qt + 1) * P, :])
                yps = py_ps.tile([P, D], F32, tag="yps")
                for half in range(2):
                    for ht in range(HT):
                        last = (ht == HT - 1) and not with_bias
                        nc.tensor.matmul(
                            yps[:, half * 512:(half + 1) * 512],
                            lhsT=ot_sb[ht][:, qt * P:(qt + 1) * P],
                            rhs=wo_sb[ht][:, half * 512:(half + 1) * 512],
                            start=(ht == 0), stop=last)
                    if with_bias:
                        nc.tensor.matmul(
                            yps[:, half * 512:(half + 1) * 512],
                            lhsT=ones_rowb[0:1, 0:P],
                            rhs=bo_sb[0:1, half * 512:(half + 1) * 512],
                            start=False, stop=True)
                # residual add (psum + sbuf -> sbuf)
                x_t = py_t.tile([P, D], F32, tag="x_t")
                nc.vector.tensor_tensor(
                    out=x_t, in0=yps, in1=xres, op=mybir.AluOpType.add)
                # mean/var in one pass via bn_stats/bn_aggr
                nsub = D // nc.vector.BN_STATS_FMAX
                stats = py_s.tile([P, nsub, nc.vector.BN_STATS_DIM], F32,
                                  tag="stats")
                xg = x_t.rearrange("p (s f) -> p s f", s=nsub)
                for s in range(nsub):
                    nc.vector.bn_stats(out=stats[:, s, :], in_=xg[:, s, :])
                mv = py_s.tile([P, nc.vector.BN_AGGR_DIM], F32, tag="mv")
                nc.vector.bn_aggr(out=mv, in_=stats)
                var_eps = py_s.tile([P, 1], F32, tag="var_eps")
                nc.vector.tensor_scalar(
                    out=var_eps, in0=mv[:, 1:2], scalar1=LN_EPS, scalar2=None,
                    op0=mybir.AluOpType.add)
                rvar = py_s.tile([P, 1], F32, tag="rvar")
                nc.vector.reciprocal(out=rvar, in_=var_eps)
                rstd = py_s.tile([P, 1], F32, tag="rstd")
                nc.scalar.sqrt(out=rstd, in_=rvar)
                xhat = py_t.tile([P, D], F32, tag="xhat")
                nc.vector.tensor_scalar(
                    out=xhat, in0=x_t, scalar1=mv[:, 0:1], scalar2=rstd,
                    op0=mybir.AluOpType.subtract, op1=mybir.AluOpType.mult)
                if with_affine:
                    yout = py_t.tile([P, D], F32, tag="yout")
                    nc.vector.tensor_tensor(
                        out=yout, in0=xhat, in1=gamma_b, op=mybir.AluOpType.mult)
                    nc.vector.tensor_tensor(
                        out=yout, in0=yout, in1=beta_b, op=mybir.AluOpType.add)
                else:
                    yout = xhat
                nc.sync.dma_start(out=y[qt * P:(qt + 1) * P, :], in_=yout)

    nc.compile()
    return nc


_NC_CACHE = {}


def _get_nc(with_bias, with_affine):
    key = (with_bias, with_affine)
    if key not in _NC_CACHE:
        _NC_CACHE[key] = build_nc(with_bias, with_affine)
    return _NC_CACHE[key]


def _kq_perm():
    """Column permutation for Wk/Wq: packed column c=(2g+s)*128+p holds
    hd index (4g + p//32)*64 + 32*s + (p%32)."""
    c = np.arange(H * DK)
    g, r = c // 256, c % 256
    s, p = r // 128, r % 128
    return (4 * g + p // 32) * 64 + 32 * s + (p % 32)


def make_in_maps(queries, keys, values, geometry, attention_mask,
                 Wq, bq, Wk, bk, Wv, bv, Wo, bo, ln_gamma, ln_beta,
                 with_bias, with_affine):
    bf16 = mybir.dt.np(BF16)
    fp8 = mybir.dt.np(FP8)
    f32 = np.float32
    perm = _kq_perm()

    def pack_w(w, dt):
        # [D, D] -> [P, DT_KT, D]: tile rows by 128, partition-major
        return np.ascontiguousarray(
            np.asarray(w, dtype=f32).reshape(DT_KT, P, D).transpose(1, 0, 2)
        ).astype(dt)

    shared = {
        "wq": pack_w(np.asarray(Wq, dtype=f32)[:, perm], fp8),
        "wk": pack_w(np.asarray(Wk, dtype=f32)[:, perm], fp8),
        "wv": pack_w(Wv, fp8),
        "wo": pack_w(Wo, bf16),
        "ones_bf": np.ones((P, H), dtype=fp8),
        "expb_in": np.full((P, 1), EXPB, dtype=f32),
        "gamma2d": np.broadcast_to(np.asarray(ln_gamma, dtype=f32), (P, D)).copy(),
        "beta2d": np.broadcast_to(np.asarray(ln_beta, dtype=f32), (P, D)).copy(),
    }
    if with_bias:
        for nm, bb, dt in (("bq_s", np.asarray(bq, dtype=f32)[perm], fp8),
                           ("bk_in", np.asarray(bk, dtype=f32)[perm], fp8),
                           ("bv_in", bv, fp8), ("bo_in", bo, bf16)):
            shared[nm] = np.asarray(bb, dtype=f32).astype(dt).reshape(1, D)
        shared["ones_row_d"] = np.ones((1, D), dtype=fp8)
        shared["ones_row_b"] = np.ones((1, D), dtype=bf16)

    # g_eff = where(mask, 0, clip(g, 1e-6)): fold the -inf mask into the
    # geometry gate (exactly equivalent post-softmax)
    g_eff = np.where(attention_mask, np.float32(0),
                     np.clip(geometry, 1e-6, None).astype(f32))  # [B,H,NQ,NK]

    def packT(x):  # [N, D] -> [DT_KT, P, cols] transposed contiguous tiles
        return np.ascontiguousarray(
            np.asarray(x, dtype=f32).T.reshape(DT_KT, P, -1)).astype(fp8)

    in_maps = []
    for c in range(NCORES):
        b, qh = c // 2, c % 2
        qs = slice(qh * NQL, (qh + 1) * NQL)
        # [H, NQL, NK] -> [H=2t+j, NK=256m+128s+p, NQL] -> [t,j,p,m,s,q]
        g_slice = g_eff[b, :, qs, :].transpose(0, 2, 1).reshape(
            HT, 2, MT, 2, P, NQL).transpose(0, 1, 4, 2, 3, 5)
        in_maps.append({
            "xqT": packT(queries[b, qs]),
            "keysT": packT(keys[b]),
            "valuesT": packT(values[b]),
            "xq": np.ascontiguousarray(queries[b, qs], dtype=f32),
            "g_t": np.ascontiguousarray(g_slice.astype(fp8)),
            **shared,
        })
    return in_maps


def kernel(queries, keys, values, geometry, attention_mask,
           Wq, bq, Wk, bk, Wv, bv, Wo, bo, ln_gamma, ln_beta, **run_kwargs):
    with_bias = bool(np.any(bq) or np.any(bk) or np.any(bv) or np.any(bo))
    with_affine = bool(np.any(np.asarray(ln_gamma) != 1) or np.any(ln_beta))
    nc = _get_nc(with_bias, with_affine)
    in_maps = make_in_maps(queries, keys, values, geometry, attention_mask,
                           Wq, bq, Wk, bk, Wv, bv, Wo, bo, ln_gamma, ln_beta,
                           with_bias, with_affine)
    res = run_bass_kernel_spmd(nc, in_maps, core_ids=list(range(NCORES)),
                               **run_kwargs)
    out = np.empty((B, NQ, D), np.float32)
    for c in range(NCORES):
        b, qh = c // 2, c % 2
        out[b, qh * NQL:(qh + 1) * NQL, :] = res.results[c]["y"]
    if run_kwargs:
        kernel.last_results = res
    return out


# revision 35
# speedup vs baseline: 9.4092x; 1.0584x over previous
"""Trainium2 Bass kernel for MultiHeadGeometryAttention.

Math (per batch b):
  q = x @ Wq + bq ; k = keys @ Wk + bk ; v = values @ Wv + bv   (per-head d=64)
  S_h = q_h k_h^T / 8
  w = softmax(log(clip(g,1e-6)) + where(mask, -inf, S))
    = g_eff * exp(S/8 - c) / rowsum(...)   with g_eff = where(mask, 0, clip(g,1e-6))
  out = (w @ v) reshaped @ Wo + bo ; y = LayerNorm(x + out) * gamma + beta

Sharding: 8 cores = 4 batches x 2 query-halves (512 q rows per core).
No collectives; K/V projections duplicated within a batch pair.

Numerics: every matmul except the score PSUM accumulation and the output
projection runs in fp8(e4m3) DoubleRow mode (two contraction tiles per PE
pass).  For the scores the d=64 contraction is packed as [32 partitions x
2 slots]; the K/Q projection weights are column-permuted on the host so
the projections write that layout directly.  exp(S/8-3) and the geometry
gate are fp8 so the PV matmul is DoubleRow over key-tile pairs; the -3
logit shift keeps exp() inside fp8 range and cancels in the softmax
normalization.  The output projection runs bf16; residual + LayerNorm f32.

All DRAM inputs are pre-packed on the host so every SBUF tile load is a
large contiguous block; input loads are hoisted so DMA runs ahead of
compute.  The attention loop is software-pipelined (S/exp/gate run AHEAD
iterations ahead of the PV accumulation so the PE never head-of-line
blocks).
"""

import numpy as np
from contextlib import ExitStack

import concourse.bass as bass
import concourse.bacc as bacc
import concourse.tile as tile
from concourse import mybir
from concourse.bass_utils import run_bass_kernel_spmd

P = 128
B, NQ, NK, D, H, DK, DV = 4, 1024, 1024, 1024, 16, 64, 64
NQL = 512           # q rows per core
NCORES = 8
EXPS = 0.125        # exp scale: logits = S/8
EXPB = -3.0         # logit shift: keeps fp8 exp() in range; cancels in softmax
LN_EPS = 1e-5

F32 = mybir.dt.float32
BF16 = mybir.dt.bfloat16
FP8 = mybir.dt.float8e4
DR = mybir.MatmulPerfMode.DoubleRow

DT_KT = 8           # D // P contraction tiles
HT = H * DK // P    # 8 head-dim tiles of 128
G4 = H // 4         # 4 groups of 4 heads (for the packed score layout)
KT_TILES = NK // P  # 8 key tiles
MT = KT_TILES // 2  # 4 key-tile pairs
QT_TILES = NQL // P # 4 query tiles
AHEAD = 2           # attention software-pipeline depth
EP_DELAY = 2        # iterations between a pair's last PV and its epilogue


def build_nc(with_bias=True, with_affine=True):
    nc = bacc.Bacc(None, target_bir_lowering=False)

    # host-packed layouts: every tile load below is contiguous in DRAM
    xqT = nc.dram_tensor("xqT", [DT_KT, P, NQL], FP8, kind="ExternalInput")
    keysT = nc.dram_tensor("keysT", [DT_KT, P, NK], FP8, kind="ExternalInput")
    valuesT = nc.dram_tensor("valuesT", [DT_KT, P, NK], FP8, kind="ExternalInput")
    xq = nc.dram_tensor("xq", [NQL, D], F32, kind="ExternalInput")
    # g gate: [t][j][p][m][slot][q]; one DMA per (t, j) covers 4 kt-pairs
    g_t = nc.dram_tensor("g_t", [HT, 2, P, MT, 2, NQL], FP8,
                         kind="ExternalInput")
    wq = nc.dram_tensor("wq", [P, DT_KT, D], FP8, kind="ExternalInput")
    wk = nc.dram_tensor("wk", [P, DT_KT, D], FP8, kind="ExternalInput")
    wv = nc.dram_tensor("wv", [P, DT_KT, D], FP8, kind="ExternalInput")
    wo = nc.dram_tensor("wo", [P, HT, D], BF16, kind="ExternalInput")
    ones_bf = nc.dram_tensor("ones_bf", [P, H], FP8, kind="ExternalInput")
    expb_in = nc.dram_tensor("expb_in", [P, 1], F32, kind="ExternalInput")
    gamma2d = nc.dram_tensor("gamma2d", [P, D], F32, kind="ExternalInput")
    beta2d = nc.dram_tensor("beta2d", [P, D], F32, kind="ExternalInput")
    if with_bias:
        bq = nc.dram_tensor("bq_s", [1, D], FP8, kind="ExternalInput")
        bk = nc.dram_tensor("bk_in", [1, D], FP8, kind="ExternalInput")
        bv = nc.dram_tensor("bv_in", [1, D], FP8, kind="ExternalInput")
        bo = nc.dram_tensor("bo_in", [1, D], BF16, kind="ExternalInput")
        ones_row_d = nc.dram_tensor("ones_row_d", [1, D], FP8,
                                    kind="ExternalInput")
        ones_row_b = nc.dram_tensor("ones_row_b", [1, D], BF16,
                                    kind="ExternalInput")
    y = nc.dram_tensor("y", [NQL, D], F32, kind="ExternalOutput")

    with tile.TileContext(nc) as tc, ExitStack() as ctx:
        persist = ctx.enter_context(tc.tile_pool(name="persist", bufs=1))

        # ---- constants + all input loads, hoisted so DMA runs early ----
        ones_hb = persist.tile([P, H], FP8, tag="ones_hb")
        nc.sync.dma_start(out=ones_hb, in_=ones_bf[:, :])
        expb_sb = persist.tile([P, 1], F32, tag="expb_sb")
        nc.sync.dma_start(out=expb_sb, in_=expb_in[:, :])
        if with_bias:
            ones_rowf = persist.tile([1, D], FP8, tag="ones_rowf")
            nc.sync.dma_start(out=ones_rowf, in_=ones_row_d[0:1, :])
            ones_rowb = persist.tile([1, D], BF16, tag="ones_rowb")
            nc.sync.dma_start(out=ones_rowb, in_=ones_row_b[0:1, :])
            bq_sb = persist.tile([1, D], FP8, tag="bq_sb")
            nc.sync.dma_start(out=bq_sb, in_=bq[0:1, :])
            bk_sb = persist.tile([1, D], FP8, tag="bk_sb")
            nc.sync.dma_start(out=bk_sb, in_=bk[0:1, :])
            bv_sb = persist.tile([1, D], FP8, tag="bv_sb")
            nc.sync.dma_start(out=bv_sb, in_=bv[0:1, :])
            bo_sb = persist.tile([1, D], BF16, tag="bo_sb")
            nc.sync.dma_start(out=bo_sb, in_=bo[0:1, :])

        vT_all = persist.tile([P, DT_KT, NK], FP8, name="vT_all")
        wv_all = persist.tile([P, DT_KT, D], FP8, name="wv_all")
        kT_all = persist.tile([P, DT_KT, NK], FP8, name="kT_all")
        wk_all = persist.tile([P, DT_KT, D], FP8, name="wk_all")
        xqT_all = persist.tile([P, DT_KT, NQL], FP8, name="xqT_all")
        wq_all = persist.tile([P, DT_KT, D], FP8, name="wq_all")
        wo_all = persist.tile([P, HT, D], BF16, name="wo_all")
        for i in range(DT_KT):
            nc.sync.dma_start(out=vT_all[:, i, :], in_=valuesT[i])
        nc.sync.dma_start(out=wv_all, in_=wv[:, :, :])
        for i in range(DT_KT):
            nc.sync.dma_start(out=kT_all[:, i, :], in_=keysT[i])
        nc.sync.dma_start(out=wk_all, in_=wk[:, :, :])
        for i in range(DT_KT):
            nc.scalar.dma_start(out=xqT_all[:, i, :], in_=xqT[i])
        nc.scalar.dma_start(out=wq_all, in_=wq[:, :, :])
        nc.scalar.dma_start(out=wo_all, in_=wo[:, :, :])
        wo_sb = [wo_all[:, i, :] for i in range(HT)]
        gamma_b = persist.tile([P, D], F32, tag="gamma_b")
        nc.scalar.dma_start(out=gamma_b, in_=gamma2d[:, :])
        beta_b = persist.tile([P, D], F32, tag="beta_b")
        nc.scalar.dma_start(out=beta_b, in_=beta2d[:, :])

        # ---- persistent activations ----
        # Vaug pairs: [m][128, 2, H*65] fp8 (per head: 64 v cols + ones col)
        vaug = [persist.tile([P, 2, H * 65], FP8, tag=f"vaug{m}",
                             name=f"vaug{m}") for m in range(MT)]
        # packed scores layout: per 4-head group g, [32a:32a+32) holds head
        # 4g+a with d = 32*slot + (p-32a)
        kt8 = [persist.tile([P, 2, NK], FP8, tag=f"kt8{g}", name=f"kt8{g}")
               for g in range(G4)]
        qt8 = [persist.tile([P, 2, NQL], FP8, tag=f"qt8{g}", name=f"qt8{g}")
               for g in range(G4)]
        # base partition 96 is not addressable by engines: head a=3 of each
        # group lives in a hopped-down copy at base 0
        ktb = [persist.tile([32, 2, NK], FP8, tag=f"ktb{g}", name=f"ktb{g}")
               for g in range(G4)]
        qtb = [persist.tile([32, 2, NQL], FP8, tag=f"qtb{g}", name=f"qtb{g}")
               for g in range(G4)]
        ot_sb = [persist.tile([P, NQL], BF16, tag=f"ot{i}", name=f"ot{i}")
                 for i in range(HT)]

        def proj_group(ps, stat_all, stat_cols, mov_all, mov_cols,
                       bias_lhsT, bias_rhs):
            """ps = sum_dt stat[:,dt,stat_cols]^T mov[:,dt,mov_cols] (+bias),
            fp8 DoubleRow over pairs of contraction tiles."""
            for dp in range(DT_KT // 2):
                last = (dp == DT_KT // 2 - 1) and not with_bias
                nc.tensor.matmul(
                    ps,
                    lhsT=stat_all[:, 2 * dp:2 * dp + 2, stat_cols],
                    rhs=mov_all[:, 2 * dp:2 * dp + 2, mov_cols],
                    start=(dp == 0), stop=last, perf_mode=DR)
            if with_bias:
                nc.tensor.matmul(ps, lhsT=bias_lhsT, rhs=bias_rhs,
                                 start=False, stop=True)

        # ================= Phase V: Vaug = values^T-proj =================
        with tc.tile_pool(name="pv_ps", bufs=2, space="PSUM") as pv_ps:
            for i in range(KT_TILES):
                m, slot = i // 2, i % 2
                nc.scalar.copy(
                    out=vaug[m][:, slot, :].rearrange(
                        "p (h c) -> p h c", c=65)[:, :, 64:65],
                    in_=ones_hb[:, :, None])
                ps = pv_ps.tile([P, 2, 512], F32, tag="vps")
                for half in range(2):
                    proj_group(
                        ps[:, half, :], vT_all, slice(i * P, (i + 1) * P),
                        wv_all, slice(half * 512, (half + 1) * 512),
                        None if not with_bias else ones_rowf[0:1, 0:P],
                        None if not with_bias else bv_sb[0:1, half * 512:(half + 1) * 512])
                nc.scalar.copy(
                    out=vaug[m][:, slot, :].rearrange(
                        "p (h c) -> p h c", c=65)[:, :, 0:64],
                    in_=ps.rearrange("p a (h c) -> p (a h) c", c=64))

        # ==== Phases K/Q + A fused: the packed-layout K/Q projection groups
        # (wk/wq host column-permuted so out partition p, group g, slot s =
        # head 4g+p//32, d = 32s+p%32) are interleaved into the attention
        # loop so the PE never idles on softmax dependencies (keeps the
        # tensor-engine p-state ramped).  They share one PSUM pool. ====
        with tc.tile_pool(name="pa_gm", bufs=3) as pa_gm, \
             tc.tile_pool(name="pa_pt", bufs=AHEAD + 2) as pa_pt, \
             tc.tile_pool(name="pa_ep", bufs=2) as pa_ep, \
             tc.tile_pool(name="pa_st", bufs=2, space="PSUM") as pa_st, \
             tc.tile_pool(name="pa_ot", bufs=1, space="PSUM") as pa_ot:

            def emit_kproj(g, s):
                c0 = (2 * g + s) * P
                ps = pa_st.tile([P, 2, 512], F32, tag="stps")
                for half in range(2):
                    proj_group(
                        ps[:, half, :], wk_all, slice(c0, c0 + P),
                        kT_all, slice(half * 512, (half + 1) * 512),
                        None if not with_bias else bk_sb[0:1, c0:c0 + P],
                        None if not with_bias else ones_rowf[0:1, 0:512])
                nc.scalar.copy(
                    out=kt8[g][:, s, :],
                    in_=ps.rearrange("p a b -> p (a b)"))
                if s == 1:
                    nc.gpsimd.dma_start(out=ktb[g], in_=kt8[g][96:128, :, :])

            def emit_qproj(g):
                ps = pa_st.tile([P, 2, NQL], F32, tag="stps")
                for s in range(2):
                    c0 = (2 * g + s) * P
                    proj_group(
                        ps[:, s, :], wq_all, slice(c0, c0 + P),
                        xqT_all, slice(0, NQL),
                        None if not with_bias else bq_sb[0:1, c0:c0 + P],
                        None if not with_bias else ones_rowf[0:1, 0:NQL])
                nc.scalar.copy(out=qt8[g], in_=ps)
                nc.gpsimd.dma_start(out=qtb[g], in_=qt8[g][96:128, :, :])

            # prelude: group 0 projections; later groups interleave into
            # the attention iteration stream below
            emit_kproj(0, 0)
            emit_kproj(0, 1)
            emit_qproj(0)
            interleave = {}
            for g in range(1, G4):
                base = (2 * g - 2) * 2 * MT  # attention iter where pair 2(g-1) starts
                interleave[base + 1] = lambda g=g: emit_kproj(g, 0)
                interleave[base + 6] = lambda g=g: emit_kproj(g, 1)
                interleave[base + 11] = lambda g=g: emit_qproj(g)

            seq = [(t, j, m) for t in range(HT) for j in range(2)
                   for m in range(MT)]
            otp = {}
            gts = {}
            pts = {}
            pend_ep = []  # [t, emit_at_index]

            def emit_front(i):
                t, j, m = seq[i]
                h = 2 * t + j
                g, a = h // 4, h % 4
                if j == 0 and m == 0:
                    otp[t] = [pa_ot.tile([65, NQL], F32, tag=f"otp{t % 2}_{jj}",
                                         name=f"otp{t}_{jj}")
                              for jj in range(2)]
                if m == 0:
                    gt = pa_gm.tile([P, MT, 2, NQL], FP8, tag="gt")
                    nc.gpsimd.dma_start(out=gt, in_=g_t[t, j])
                    gts[(t, j)] = gt
                # two DoubleRow S matmuls (kt = 2m, 2m+1), packed d=[32x2]
                stp = pa_st.tile([P, 2, NQL], F32, tag="stps")
                for s in range(2):
                    kt = 2 * m + s
                    if a == 3:
                        kl = ktb[g][:, :, kt * P:(kt + 1) * P]
                        qr = qtb[g][:, :, :]
                    else:
                        kl = kt8[g][32 * a:32 * a + 32, :, kt * P:(kt + 1) * P]
                        qr = qt8[g][32 * a:32 * a + 32, :, :]
                    nc.tensor.matmul(
                        stp[:, s, :], lhsT=kl, rhs=qr,
                        start=True, stop=True, perf_mode=DR)
                pt0 = pa_pt.tile([P, 2, NQL], FP8, tag="pt0")
                nc.scalar.activation(
                    out=pt0, in_=stp,
                    func=mybir.ActivationFunctionType.Exp,
                    scale=EXPS, bias=expb_sb[:, 0:1])
                pt = pa_pt.tile([P, 2, NQL], FP8, tag="pt")
                nc.vector.tensor_tensor(
                    out=pt, in0=pt0, in1=gts[(t, j)][:, m, :, :],
                    op=mybir.AluOpType.mult)
                pts[(t, j, m)] = pt

            def emit_pv(i):
                t, j, m = seq[i]
                h = 2 * t + j
                nc.tensor.matmul(
                    otp[t][j],
                    lhsT=vaug[m][:, :, h * 65:(h + 1) * 65],
                    rhs=pts.pop((t, j, m)),
                    start=(m == 0), stop=(m == MT - 1), perf_mode=DR)
                if m == MT - 1 and j == 1:
                    pend_ep.append([t, i + EP_DELAY])

            def emit_epilogue(t):
                # gather both rowsum rows -> one reciprocal -> gpsimd
                # partition-broadcast -> divide into ot_sb
                r2 = pa_ep.tile([2, NQL], F32, tag="r2")
                for j in range(2):
                    rs = pa_ep.tile([P, NQL], F32, tag=f"rs{j}")
                    nc.scalar.copy(out=rs[64:65, :], in_=otp[t][j][64:65, :])
                    nc.gpsimd.dma_start(out=r2[j:j + 1, :], in_=rs[64:65, :])
                rinv2 = pa_ep.tile([2, NQL], F32, tag="rinv2")
                nc.vector.reciprocal(out=rinv2, in_=r2)
                rinv1 = pa_ep.tile([1, NQL], F32, tag="rinv1")
                nc.gpsimd.dma_start(out=rinv1, in_=rinv2[1:2, :])
                for j in range(2):
                    rb = pa_ep.tile([64, NQL], F32, tag=f"rb{j}")
                    nc.gpsimd.partition_broadcast(
                        out_ap=rb, in_ap=rinv2[0:1, :] if j == 0 else rinv1)
                    if j == 0:
                        nc.vector.tensor_tensor(
                            out=ot_sb[t][0:64, :], in0=otp[t][j][0:64, :],
                            in1=rb, op=mybir.AluOpType.mult)
                    else:
                        tmp = pa_ep.tile([64, NQL], BF16, tag="ottmp")
                        nc.vector.tensor_tensor(
                            out=tmp, in0=otp[t][j][0:64, :], in1=rb,
                            op=mybir.AluOpType.mult)
                        # partition shift 0-63 -> 64-127 needs a DMA hop
                        nc.gpsimd.dma_start(out=ot_sb[t][64:128, :], in_=tmp)

            n = len(seq)
            for i in range(n + AHEAD):
                if i in interleave:
                    interleave.pop(i)()
                if i < n:
                    emit_front(i)
                if i >= AHEAD:
                    emit_pv(i - AHEAD)
                while pend_ep and pend_ep[0][1] <= i:
                    emit_epilogue(pend_ep.pop(0)[0])
            while pend_ep:
                emit_epilogue(pend_ep.pop(0)[0])

        # ================= Phase Y: out proj + residual + LN =================
        with tc.tile_pool(name="py_x", bufs=2) as py_x, \
             tc.tile_pool(name="py_t", bufs=2) as py_t, \
             tc.tile_pool(name="py_s", bufs=4) as py_s, \
             tc.tile_pool(name="py_ps", bufs=2, space="PSUM") as py_ps:
            for qt in range(QT_TILES):
                xres = py_x.tile([P, D], F32, tag="xres")
                nc.sync.dma_start(out=xres, in_=xq[qt * P:(qt + 1) * P, :])
                yps = py_ps.tile([P, D], F32, tag="yps")
                for half in range(2):
                    for ht in range(HT):
                        last = (ht == HT - 1) and not with_bias
                        nc.tensor.matmul(
                            yps[:, half * 512:(half + 1) * 512],
                            lhsT=ot_sb[ht][:, qt * P:(qt + 1) * P],
                            rhs=wo_sb[ht][:, half * 512:(half + 1) * 512],
                            start=(ht == 0), stop=last)
                    if with_bias:
                        nc.tensor.matmul(
                            yps[:, half * 512:(half + 1) * 512],
                            lhsT=ones_rowb[0:1, 0:P],
                            rhs=bo_sb[0:1, half * 512:(half + 1) * 512],
                            start=False, stop=True)
                # residual add (psum + sbuf -> sbuf)
                x_t = py_t.tile([P, D], F32, tag="x_t")
                nc.vector.tensor_tensor(
                    out=x_t, in0=yps, in1=xres, op=mybir.AluOpType.add)
                # mean/var in one pass via bn_stats/bn_aggr
                nsub = D // nc.vector.BN_STATS_FMAX
                stats = py_s.tile([P, nsub, nc.vector.BN_STATS_DIM], F32,
                                  tag="stats")
                xg = x_t.rearrange("p (s f) -> p s f", s=nsub)
                for s in range(nsub):
                    nc.vector.bn_stats(out=stats[:, s, :], in_=xg[:, s, :])
                mv = py_s.tile([P, nc.vector.BN_AGGR_DIM], F32, tag="mv")
                nc.vector.bn_aggr(out=mv, in_=stats)
                var_eps = py_s.tile([P, 1], F32, tag="var_eps")
                nc.vector.tensor_scalar(
                    out=var_eps, in0=mv[:, 1:2], scalar1=LN_EPS, scalar2=None,
                    op0=mybir.AluOpType.add)
                rvar = py_s.tile([P, 1], F32, tag="rvar")
                nc.vector.reciprocal(out=rvar, in_=var_eps)
                rstd = py_s.tile([P, 1], F32, tag="rstd")
                nc.scalar.sqrt(out=rstd, in_=rvar)
                xhat = py_t.tile([P, D], F32, tag="xhat")
                nc.vector.tensor_scalar(
                    out=xhat, in0=x_t, scalar1=mv[:, 0:1], scalar2=rstd,
                    op0=mybir.AluOpType.subtract, op1=mybir.AluOpType.mult)
                if with_affine:
                    yout = py_t.tile([P, D], F32, tag="yout")
                    nc.vector.tensor_tensor(
                        out=yout, in0=xhat, in1=gamma_b, op=mybir.AluOpType.mult)
                    nc.vector.tensor_tensor(
                        out=yout, in0=yout, in1=beta_b, op=mybir.AluOpType.add)
                else:
                    yout = xhat
                nc.sync.dma_start(out=y[qt * P:(qt + 1) * P, :], in_=yout)

    nc.compile()
    return nc


_NC_CACHE = {}


def _get_nc(with_bias, with_affine):
    key = (with_bias, with_affine)
    if key not in _NC_CACHE:
        _NC_CACHE[key] = build_nc(with_bias, with_affine)
    return _NC_CACHE[key]


def _kq_perm():
    """Column permutation for Wk/Wq: packed column c=(2g+s)*128+p holds
    hd index (4g + p//32)*64 + 32*s + (p%32)."""
    c = np.arange(H * DK)
    g, r = c // 256, c % 256
    s, p = r // 128, r % 128
    return (4 * g + p // 32) * 64 + 32 * s + (p % 32)


def make_in_maps(queries, keys, values, geometry, attention_mask,
                 Wq, bq, Wk, bk, Wv, bv, Wo, bo, ln_gamma, ln_beta,
                 with_bias, with_affine):
    bf16 = mybir.dt.np(BF16)
    fp8 = mybir.dt.np(FP8)
    f32 = np.float32
    perm = _kq_perm()

    def pack_w(w, dt):
        # [D, D] -> [P, DT_KT, D]: tile rows by 128, partition-major
        return np.ascontiguousarray(
            np.asarray(w, dtype=f32).reshape(DT_KT, P, D).transpose(1, 0, 2)
        ).astype(dt)

    shared = {
        "wq": pack_w(np.asarray(Wq, dtype=f32)[:, perm], fp8),
        "wk": pack_w(np.asarray(Wk, dtype=f32)[:, perm], fp8),
        "wv": pack_w(Wv, fp8),
        "wo": pack_w(Wo, bf16),
        "ones_bf": np.ones((P, H), dtype=fp8),
        "expb_in": np.full((P, 1), EXPB, dtype=f32),
        "gamma2d": np.broadcast_to(np.asarray(ln_gamma, dtype=f32), (P, D)).copy(),
        "beta2d": np.broadcast_to(np.asarray(ln_beta, dtype=f32), (P, D)).copy(),
    }
    if with_bias:
        for nm, bb, dt in (("bq_s", np.asarray(bq, dtype=f32)[perm], fp8),
                           ("bk_in", np.asarray(bk, dtype=f32)[perm], fp8),
                           ("bv_in", bv, fp8), ("bo_in", bo, bf16)):
            shared[nm] = np.asarray(bb, dtype=f32).astype(dt).reshape(1, D)
        shared["ones_row_d"] = np.ones((1, D), dtype=fp8)
        shared["ones_row_b"] = np.ones((1, D), dtype=bf16)

    # g_eff = where(mask, 0, clip(g, 1e-6)): fold the -inf mask into the
    # geometry gate (exactly equivalent post-softmax)
    g_eff = np.where(attention_mask, np.float32(0),
                     np.clip(geometry, 1e-6, None).astype(f32))  # [B,H,NQ,NK]

    def packT(x):  # [N, D] -> [DT_KT, P, cols] transposed contiguous tiles
        return np.ascontiguousarray(
            np.asarray(x, dtype=f32).T.reshape(DT_KT, P, -1)).astype(fp8)

    in_maps = []
    for c in range(NCORES):
        b, qh = c // 2, c % 2
        qs = slice(qh * NQL, (qh + 1) * NQL)
        # [H, NQL, NK] -> [H=2t+j, NK=256m+128s+p, NQL] -> [t,j,p,m,s,q]
        g_slice = g_eff[b, :, qs, :].transpose(0, 2, 1).reshape(
            HT, 2, MT, 2, P, NQL).transpose(0, 1, 4, 2, 3, 5)
        in_maps.append({
            "xqT": packT(queries[b, qs]),
            "keysT": packT(keys[b]),
            "valuesT": packT(values[b]),
            "xq": np.ascontiguousarray(queries[b, qs], dtype=f32),
            "g_t": np.ascontiguousarray(g_slice.astype(fp8)),
            **shared,
        })
    return in_maps


def kernel(queries, keys, values, geometry, attention_mask,
           Wq, bq, Wk, bk, Wv, bv, Wo, bo, ln_gamma, ln_beta, **run_kwargs):
    with_bias = bool(np.any(bq) or np.any(bk) or np.any(bv) or np.any(bo))
    with_affine = bool(np.any(np.asarray(ln_gamma) != 1) or np.any(ln_beta))
    nc = _get_nc(with_bias, with_affine)
    in_maps = make_in_maps(queries, keys, values, geometry, attention_mask,
                           Wq, bq, Wk, bk, Wv, bv, Wo, bo, ln_gamma, ln_beta,
                           with_bias, with_affine)
    res = run_bass_kernel_spmd(nc, in_maps, core_ids=list(range(NCORES)),
                               **run_kwargs)
    out = np.empty((B, NQ, D), np.float32)
    for c in range(NCORES):
        b, qh = c // 2, c % 2
        out[b, qh * NQL:(qh + 1) * NQL, :] = res.results[c]["y"]
    if run_kwargs:
        kernel.last_results = res
    return out
